# revision 21
# baseline (speedup 1.0000x reference)
"""Trainium2 Bass kernel for nn_Attention_32650341384246.

Full attention layer: qkv proj + per-head RMSNorm(q,k) + RoPE + softmax
attention (non-causal) + out proj.  B=2, S=2048, D=1024, H=16, DH=64.

Sharding: 8 cores; core c handles batch c//4, heads [4*(c%4), 4*(c%4)+4)
(data parallel over batch x tensor parallel over heads).  Each core
computes a partial [S, D] output (its heads @ Wout row-slice); the host
sums the 4 partials per batch and adds the (folded) biases.

Device design (per core), v2 (ACT-exp roofline shape):
  - qkv proj emits qT/kT head-major [128 (2 heads x 64), S] (lhsT = W
    slice, rhs = xT slice) and v s-major [s, 4*64].
  - q-path RMSNorm: sumsq via ones-block matmul into a shared [8,512]
    PSUM tile (one Ln+Exp per section), partition-broadcast via
    ones-matmul, u = tt*pb on DVE.
  - k-path RMSNorm is FOLDED INTO THE SOFTMAX EXP: sumsq is computed
    transposed ([s-part, head] via lhsT=sq-slice matmuls into [128,32]
    PSUM), one Ln+Exp gives rsT = exp_scale*rsqrt(ms+eps), and the
    attention exp uses scale=rsT[:, 2*kt+h] (per-partition AP).  The
    k rope output is left unnormalized (rsqrt commutes past rope).
  - RoPE as rot = cosT*u + sinT'*swap(u); swap = adjacent-partition
    permutation matmul (bf16); tables host-built with scales folded.
  - attention: 8 sections (qh, pair, h); per kt: one [128,1024] score
    matmul (double-buffered PSUM scA/scB), one [128,1024] exp, one
    accumulating [65,1024] AV matmul (row 64 = ones = sumexp).  The
    emission is a flat software-pipelined stream: av lags sc/exp by 2
    slots, section normalize (reciprocal + K=1 broadcast matmul +
    vmix mul) is deferred into the next section's slots, so ACT stays
    saturated across section boundaries.
  - out proj: interleaved into the last attention sections (PSUM tag
    shared with the normalize broadcast), copies on DVE, bf16 out.
ACT runs ONLY Exp/Ln (single table set, no reloads).
"""
import sys, os

sys.path.insert(0, "/opt/trn_rl_repo")

import math
import numpy as np
from contextlib import ExitStack

import ml_dtypes
import concourse.bass as bass
import concourse.mybir as mybir
import concourse.tile as tile
from concourse import bacc
from concourse import bass_utils

F32 = mybir.dt.float32
F32R = mybir.dt.float32r
BF16 = mybir.dt.bfloat16
AF = mybir.ActivationFunctionType

B, S, DM, H, DH = 2, 2048, 1024, 16, 64
NC = 8
HPC = H // 4          # 4 heads per core
HD = HPC * DH         # 256
NDT = DM // 128       # 8 model-dim tiles
THETA, EPS = 10000.0, 1e-6

LAST_RESULTS = None   # BassKernelResults of the most recent device run
_CACHED = {}
DEBUG = False         # add debug DMA taps (qt/kt/rsT) for CoreSim bisection


def build_program(exp_scale: float, shared_tables: bool):
    nc = bacc.Bacc("TRN2", target_bir_lowering=False, debug=False)

    xT_d = nc.dram_tensor("xT", [128, NDT, S], BF16, kind="ExternalInput")
    w_d = nc.dram_tensor("w_all", [128, NDT, 3 * HD], BF16, kind="ExternalInput")
    wout_d = nc.dram_tensor("wout", [128, 2, DM], BF16, kind="ExternalInput")
    bq_d = nc.dram_tensor("bq", [128, 2], F32, kind="ExternalInput")
    bk_d = nc.dram_tensor("bk", [128, 2], F32, kind="ExternalInput")
    cosk_d = nc.dram_tensor("cos_k", [128, S], F32, kind="ExternalInput")
    sink_d = nc.dram_tensor("sin_k", [128, S], F32, kind="ExternalInput")
    if not shared_tables:
        cosq_d = nc.dram_tensor("cos_q", [128, S], F32, kind="ExternalInput")
        sinq_d = nc.dram_tensor("sin_q", [128, S], F32, kind="ExternalInput")
    P_d = nc.dram_tensor("Pswap", [128, 128], BF16, kind="ExternalInput")
    ob_d = nc.dram_tensor("onesblk", [128, 2], BF16, kind="ExternalInput")
    ob8_d = nc.dram_tensor("onesblk8", [128, 4, 8], BF16, kind="ExternalInput")
    o2b8_d = nc.dram_tensor("ones2blk8", [8, 4, 128], F32R, kind="ExternalInput")
    sel2_d = nc.dram_tensor("sel2", [1, 2, 128], F32R, kind="ExternalInput")
    out_d = nc.dram_tensor("outp", [S, DM], BF16, kind="ExternalOutput")

    ln_exp_bias = float(math.log(exp_scale)) if exp_scale != 1.0 else 0.0

    with tile.TileContext(nc) as tc, ExitStack() as ctx, \
            nc.allow_low_precision(reason="bf16 matmul inputs"):
        singles = ctx.enter_context(tc.tile_pool(name="singles", bufs=1))
        tmp = ctx.enter_context(tc.tile_pool(name="tmp", bufs=2))
        expp = ctx.enter_context(tc.tile_pool(name="expp", bufs=2))
        outp = ctx.enter_context(tc.tile_pool(name="outp", bufs=4))

        # --- first-needed loads up front; per-dt tiles so Tile's
        # per-tile RAW tracking doesn't serialize readers behind all DMAs ---
        w_dt = [singles.tile([128, 3 * HD], BF16, name=f"w{dt}") for dt in range(NDT)]
        x_dt = [singles.tile([128, S], BF16, name=f"x{dt}") for dt in range(NDT)]
        for dt in range(NDT):
            nc.sync.dma_start(out=w_dt[dt], in_=w_d.ap()[:, dt, :])
            nc.sync.dma_start(out=x_dt[dt], in_=xT_d.ap()[:, dt, :])

        wout = singles.tile([128, 2, DM], BF16)
        nc.sync.dma_start(out=wout, in_=wout_d.ap())
        bq = singles.tile([128, 2], F32)
        nc.sync.dma_start(out=bq, in_=bq_d.ap())
        bk = singles.tile([128, 2], F32)
        nc.sync.dma_start(out=bk, in_=bk_d.ap())
        cos_k = singles.tile([128, S], F32)
        nc.sync.dma_start(out=cos_k, in_=cosk_d.ap())
        sin_k = singles.tile([128, S], F32)
        nc.sync.dma_start(out=sin_k, in_=sink_d.ap())
        if shared_tables:
            cos_q, sin_q = cos_k, sin_k
        else:
            cos_q = singles.tile([128, S], F32)
            nc.sync.dma_start(out=cos_q, in_=cosq_d.ap())
            sin_q = singles.tile([128, S], F32)
            nc.sync.dma_start(out=sin_q, in_=sinq_d.ap())
        Pm = singles.tile([128, 128], BF16)
        nc.sync.dma_start(out=Pm, in_=P_d.ap())
        onesblk = singles.tile([128, 2], BF16)
        nc.sync.dma_start(out=onesblk, in_=ob_d.ap())
        onesblk8 = singles.tile([128, 4, 8], BF16)
        nc.sync.dma_start(out=onesblk8, in_=ob8_d.ap())
        ones2blk8 = singles.tile([8, 4, 128], F32R)
        nc.sync.dma_start(out=ones2blk8, in_=o2b8_d.ap())
        sel2 = singles.tile([1, 2, 128], F32R)
        nc.sync.dma_start(out=sel2, in_=sel2_d.ap())
        eps8 = singles.tile([8, 1], F32)
        nc.vector.memset(eps8, EPS)
        eps128 = singles.tile([128, 1], F32)
        nc.vector.memset(eps128, EPS)
        lnb128 = singles.tile([128, 1], F32)
        nc.vector.memset(lnb128, ln_exp_bias)

        # head-major roped q/k: [128 (2 heads x 64 dims), 2048 s] per pair
        qt = [singles.tile([128, S], BF16, name=f"qt{t}") for t in range(2)]
        kt_ = [singles.tile([128, S], BF16, name=f"kt{t}") for t in range(2)]
        # rsT[t][:, 2*kt+h] = exp_scale * rsqrt(mean k^2 + eps) per k-pos
        rsT = [singles.tile([128, 32], F32, name=f"rsT{t}") for t in range(2)]
        vhat = [singles.tile([128, 4, HPC, 65], BF16, name=f"vhat{sc}")
                for sc in range(4)]
        for sc in range(4):
            nc.vector.memset(vhat[sc][:, :, :, 64:65], 1.0)
        vmix = [[singles.tile([128, 1024], BF16, name=f"vmix{t}_{qh}")
                 for qh in range(2)] for t in range(2)]

        # ---------------- phase 1: qkv + rmsnorm + rope ----------------
        with tc.tile_pool(name="ps1", bufs=1, space="PSUM") as ps1:
            def emit_A(which, t):
                """proj + bias + square + sumsq (+ k: transposed rsqrt)."""
                off = 0 if which == "q" else HD
                bias = bq if which == "q" else bk
                tts = []
                if which == "q":
                    pss = ps1.tile([8, 512], F32, tag="pss",
                                   name=f"pss{which}{t}")
                else:
                    rstp = ps1.tile([128, 32], F32, tag="rstp",
                                    name=f"rstp{t}")
                for sc in range(4):
                    s0 = sc * 512
                    pq = ps1.tile([128, 512], F32, tag="pq", bufs=2,
                                  name=f"pq{which}{t}_{sc}")
                    for dt in range(NDT):
                        nc.tensor.matmul(
                            pq[:, :],
                            w_dt[dt][:, off + t * 128: off + (t + 1) * 128],
                            x_dt[dt][:, s0:s0 + 512],
                            start=(dt == 0), stop=(dt == NDT - 1))
                    tt = tmp.tile([128, 512], BF16, tag="tt", bufs=10,
                                  name=f"tt{which}{t}_{sc}")
                    nc.vector.tensor_scalar_add(tt[:, :], pq[:, :],
                                                bias[:, t:t + 1])
                    tts.append(tt)
                    sq = tmp.tile([128, 512], BF16, tag="sq", bufs=3,
                                  name=f"sq{which}{t}_{sc}")
                    nc.vector.tensor_mul(sq[:, :], tt[:, :], tt[:, :])
                    if which == "q":
                        nc.tensor.matmul(pss[:, :], onesblk8[:, sc, :],
                                         sq[:, :],
                                         start=(sc == 0), stop=(sc == 3))
                    else:
                        for qtr in range(4):
                            kt = 4 * sc + qtr
                            nc.tensor.matmul(
                                rstp[:, 2 * kt:2 * kt + 2],
                                sq[:, qtr * 128:(qtr + 1) * 128],
                                onesblk[:, :],
                                start=True, stop=True)
                if which == "q":
                    lns = tmp.tile([8, 512], F32, tag="lns",
                                   name=f"lns{which}{t}")
                    nc.scalar.activation(lns[:, :], pss[:, :], AF.Ln,
                                         bias=eps8[:, :], scale=1.0 / DH)
                    rs = tmp.tile([8, 512], F32R, tag="rs",
                                  name=f"rs{which}{t}")
                    nc.scalar.activation(rs[:, :], lns[:, :], AF.Exp,
                                         scale=-0.5)
                    return tts, rs
                else:
                    lnt = tmp.tile([128, 32], F32, tag="lnt",
                                   name=f"lnt{t}")
                    nc.scalar.activation(lnt[:, :], rstp[:, :], AF.Ln,
                                         bias=eps128[:, :], scale=1.0 / DH)
                    nc.scalar.activation(rsT[t][:, :], lnt[:, :], AF.Exp,
                                         scale=-0.5, bias=lnb128[:, :])
                    return tts, None

            def emit_B(which, t, tts, rs):
                """(q: normalize) + rope -> qt/kt tiles."""
                cosT = cos_q if which == "q" else cos_k
                sinT = sin_q if which == "q" else sin_k
                dest = qt[t] if which == "q" else kt_[t]
                for sc in range(4):
                    s0 = sc * 512
                    if which == "q":
                        pb = ps1.tile([128, 512], F32, tag="pb",
                                      name=f"pb{which}{t}_{sc}")
                        nc.tensor.matmul(pb[:, :], ones2blk8[:, sc, :],
                                         rs[:, :], start=True, stop=True)
                        u = tmp.tile([128, 512], BF16, tag="u", bufs=3,
                                     name=f"u{which}{t}_{sc}")
                        nc.vector.tensor_mul(u[:, :], tts[sc][:, :], pb[:, :])
                    else:
                        u = tts[sc]
                    psw = ps1.tile([128, 512], F32, tag="psw",
                                   name=f"psw{which}{t}_{sc}")
                    nc.tensor.matmul(psw[:, :], Pm[:, :], u[:, :],
                                     start=True, stop=True)
                    t1 = tmp.tile([128, 512], F32, tag="t1", bufs=3,
                                  name=f"t1{which}{t}_{sc}")
                    nc.vector.tensor_mul(t1[:, :], u[:, :],
                                         cosT[:, s0:s0 + 512])
                    t2 = tmp.tile([128, 512], F32, tag="t2", bufs=3,
                                  name=f"t2{which}{t}_{sc}")
                    nc.vector.tensor_mul(t2[:, :], psw[:, :],
                                         sinT[:, s0:s0 + 512])
                    nc.vector.tensor_add(dest[:, s0:s0 + 512],
                                         t1[:, :], t2[:, :])

            def emit_V():
                for sc in range(4):
                    for st in range(4):
                        pv = ps1.tile([128, HD], F32, tag="pv", bufs=2,
                                      name=f"pv{sc}_{st}")
                        for dt in range(NDT):
                            nc.tensor.matmul(
                                pv[:, :],
                                x_dt[dt][:, sc * 512 + st * 128:
                                         sc * 512 + (st + 1) * 128],
                                w_dt[dt][:, 2 * HD:3 * HD],
                                start=(dt == 0), stop=(dt == NDT - 1))
                        nc.vector.tensor_copy(
                            vhat[sc][:, st, :, 0:64],
                            pv[:, :].rearrange("p (h d) -> p h d", h=HPC))

            # A/B software pipeline: B(x) consumes rsqrt computed during
            # the next A's matmuls, so PE never stalls on ACT.
            a_k0 = emit_A("k", 0)
            a_q0 = emit_A("q", 0)
            emit_B("k", 0, *a_k0)
            a_k1 = emit_A("k", 1)
            emit_B("q", 0, *a_q0)
            a_q1 = emit_A("q", 1)
            emit_B("k", 1, *a_k1)
            emit_B("q", 1, *a_q1)
            emit_V()

        if DEBUG:
            for t in range(2):
                for nm, src in ((f"dbg_qt{t}", qt[t]), (f"dbg_kt{t}", kt_[t])):
                    d = nc.dram_tensor(nm, [128, S], BF16,
                                       kind="ExternalOutput")
                    nc.sync.dma_start(out=d.ap(), in_=src[:, :])
                d = nc.dram_tensor(f"dbg_rsT{t}", [128, 32], F32,
                                   kind="ExternalOutput")
                nc.sync.dma_start(out=d.ap(), in_=rsT[t][:, :])

        # ---------------- phase 2: attention (+ interleaved out proj) ---
        with tc.tile_pool(name="ps2", bufs=1, space="PSUM") as ps2:
            from collections import deque
            avq = deque()       # pending av / normalize emitters
            po_units = deque()  # pending out-proj emitters

            def make_po(st):
                def emit():
                    qh = st // 8
                    po = ps2.tile([128, 1024], F32, tag="nb",
                                  name=f"po{st}")
                    for qc in range(2):
                        for t in range(2):
                            nc.tensor.matmul(
                                po[:, qc * 512:(qc + 1) * 512],
                                vmix[t][qh][:, (st % 8) * 128:(st % 8 + 1) * 128],
                                wout[:, t, qc * 512:(qc + 1) * 512],
                                start=(t == 0), stop=(t == 1))
                    o = outp.tile([128, 1024], BF16, tag="o", name=f"o{st}")
                    nc.vector.tensor_copy(o[:, :], po[:, :])
                    nc.sync.dma_start(
                        out=out_d.ap()[st * 128:(st + 1) * 128, :],
                        in_=o[:, :])
                return emit

            sections = [(pair, h, qh) for qh in range(2)
                        for pair in range(2) for h in range(2)]
            for si, (pair, h, qh) in enumerate(sections):
                q0 = qh * 1024
                head = 2 * pair + h
                ps_sc = [ps2.tile([128, 1024], F32, tag=f"sc{ab}",
                                  name=f"sc{si}_{ab}") for ab in range(2)]
                pav = ps2.tile([65, 1024], F32, tag="av", name=f"av{si}")
                es = {}
                for kt in range(16):
                    pssc = ps_sc[kt % 2]
                    for qc in range(2):
                        nc.tensor.matmul(
                            pssc[:, qc * 512:(qc + 1) * 512],
                            kt_[pair][h * 64:(h + 1) * 64,
                                      kt * 128:(kt + 1) * 128],
                            qt[pair][h * 64:(h + 1) * 64,
                                     q0 + qc * 512:q0 + (qc + 1) * 512],
                            start=True, stop=True, tile_position=(h * 64, 0))
                    e = expp.tile([128, 1024], BF16, tag=f"e{kt % 4}",
                                  name=f"e{si}_{kt}")
                    nc.scalar.activation(
                        e[:, :], pssc[:, :], AF.Exp,
                        scale=rsT[pair][:, 2 * kt + h:2 * kt + h + 1])
                    es[kt] = e
                    # drain one pending unit (av lags by ~2 slots)
                    if kt == 6 and po_units:
                        po_units.popleft()()
                    if avq:
                        avq.popleft()()
                        if kt == 3 and avq:   # catch up after normalize slot
                            avq.popleft()()
                    if kt == 11 and po_units:
                        po_units.popleft()()

                    def make_av(kt, pav=pav, es=es, head=head, si=si):
                        def emit():
                            ek = es[kt]
                            for qc in range(2):
                                nc.tensor.matmul(
                                    pav[:, qc * 512:(qc + 1) * 512],
                                    vhat[kt // 4][:, kt % 4, head, :],
                                    ek[:, qc * 512:(qc + 1) * 512],
                                    start=(kt == 0), stop=(kt == 15),
                                    skip_group_check=True)
                        return emit
                    if kt >= 2:
                        make_av(kt - 2)()
                # leftovers: av(14), av(15), then normalize
                avq.append(make_av(14))
                avq.append(make_av(15))

                def make_norm(pair=pair, h=h, qh=qh, pav=pav, si=si):
                    def emit():
                        se = tmp.tile([1, 1024], F32, tag="se",
                                      name=f"se{si}")
                        nc.vector.tensor_copy(se[:, :], pav[64:65, :])
                        rc = tmp.tile([1, 1024], F32R, tag="rc",
                                      name=f"rc{si}")
                        from concourse.dve_ops import (
                            RECIP_APPROX_FAST_CONSTS, RECIPROCAL_APPROX_FAST)
                        _c = RECIP_APPROX_FAST_CONSTS
                        nc.vector._custom_dve(RECIPROCAL_APPROX_FAST,
                                              out=rc[:, :],
                                              in0=se[:, :],
                                              s0=_c["s0"], s1=_c["s1"],
                                              imm2=_c["imm2"])
                        nb = ps2.tile([128, 1024], F32, tag="nb",
                                      name=f"nb{si}")
                        for qc in range(2):
                            nc.tensor.matmul(nb[:, qc * 512:(qc + 1) * 512],
                                             sel2[:, h, :],
                                             rc[:, qc * 512:(qc + 1) * 512],
                                             start=True, stop=True)
                        avs = tmp.tile([64, 1024], F32, tag="avs",
                                       name=f"avs{si}")
                        nc.vector.tensor_copy(avs[:, :], pav[0:64, :])
                        nc.vector.tensor_mul(
                            vmix[pair][qh][h * 64:(h + 1) * 64, :],
                            avs[:, :], nb[h * 64:(h + 1) * 64, :])
                    return emit
                avq.append(make_norm())
                if si == 3:      # vmix[*][qh0] complete after section 3
                    for st in range(8):
                        po_units.append(make_po(st))

            # tail: drain remaining av/normalize, then out proj qh1
            while avq:
                avq.popleft()()
            for st in range(8, 16):
                po_units.append(make_po(st))
            while po_units:
                po_units.popleft()()

            if DEBUG:
                for t in range(2):
                    for qh in range(2):
                        d = nc.dram_tensor(f"dbg_vmix{t}{qh}", [128, 1024],
                                           BF16, kind="ExternalOutput")
                        nc.sync.dma_start(out=d.ap(), in_=vmix[t][qh][:, :])

    nc.compile()
    return nc


def host_prep(x, pos, Wqkv, bqkv, Wout, bout, q_scale, k_scale):
    """Build per-core input maps + shared-table decision."""
    x = np.asarray(x, dtype=np.float32)
    pos = np.asarray(pos, dtype=np.float32).reshape(-1)
    Wqkv = np.asarray(Wqkv, dtype=np.float32)
    bqkv = np.asarray(bqkv, dtype=np.float32)
    Wout = np.asarray(Wout, dtype=np.float32)
    q_scale = np.asarray(q_scale, dtype=np.float32)
    k_scale = np.asarray(k_scale, dtype=np.float32)

    shared = bool(np.array_equal(q_scale, k_scale))
    exp_scale = (1.0 / np.sqrt(DH)) if shared else 1.0

    # rope base tables [128, S]
    i_of_p = (np.arange(128) % 64) // 2            # pair index
    sign = np.where(np.arange(128) % 2 == 0, 1.0, -1.0)
    omega = THETA ** (-np.arange(0, DH, 2, dtype=np.float64) / DH)  # [32]
    ang = pos[None, :].astype(np.float64) * omega[:, None]          # [32, S]
    cosb = np.cos(ang)[i_of_p, :]                  # [128, S]
    sinb = np.sin(ang)[i_of_p, :] * sign[:, None]

    def tables(scale_vec, extra):
        sv = np.tile(scale_vec, 2)                 # [128]
        svx = np.tile(scale_vec[np.arange(64) ^ 1], 2)
        cosT = (cosb * sv[:, None] * extra).astype(np.float32)
        sinT = (sinb * svx[:, None] * extra).astype(np.float32)
        return np.ascontiguousarray(cosT), np.ascontiguousarray(sinT)

    cos_k, sin_k = tables(k_scale, 1.0)
    if not shared:
        cos_q, sin_q = tables(q_scale, 1.0 / np.sqrt(DH))

    bf = ml_dtypes.bfloat16
    Pm = np.zeros((128, 128), dtype=bf)
    Pm[np.arange(128), np.arange(128) ^ 1] = 1.0
    onesblk = np.zeros((128, 2), dtype=bf)
    onesblk[0:64, 0] = 1.0
    onesblk[64:128, 1] = 1.0
    # q-path sumsq gather: [8,512] rows (2*sc, 2*sc+1) = head halves
    onesblk8 = np.zeros((128, 4, 8), dtype=bf)
    ones2blk8 = np.zeros((8, 4, 128), dtype=np.float32)
    for sc in range(4):
        onesblk8[0:64, sc, 2 * sc] = 1.0
        onesblk8[64:128, sc, 2 * sc + 1] = 1.0
        ones2blk8[2 * sc, sc, 0:64] = 1.0
        ones2blk8[2 * sc + 1, sc, 64:128] = 1.0
    sel2 = np.zeros((1, 2, 128), dtype=np.float32)
    sel2[0, 0, 0:64] = 1.0
    sel2[0, 1, 64:128] = 1.0

    in_maps = []
    for c in range(NC):
        b, g = c // 4, c % 4
        xT = np.ascontiguousarray(
            x[b].T.reshape(NDT, 128, S).transpose(1, 0, 2)).astype(bf)
        wq = Wqkv[:, g * HD:(g + 1) * HD]
        wk = Wqkv[:, DM + g * HD: DM + (g + 1) * HD]
        wv = Wqkv[:, 2 * DM + g * HD: 2 * DM + (g + 1) * HD]
        w_all = np.ascontiguousarray(
            np.concatenate([wq, wk, wv], axis=1)
            .reshape(NDT, 128, 3 * HD).transpose(1, 0, 2)).astype(bf)
        wo = np.ascontiguousarray(
            Wout[g * HD:(g + 1) * HD, :]
            .reshape(2, 128, DM).transpose(1, 0, 2)).astype(bf)
        bqs = np.ascontiguousarray(
            bqkv[g * HD:(g + 1) * HD].reshape(2, 128).T)         # [128, 2]
        bks = np.ascontiguousarray(
            bqkv[DM + g * HD: DM + (g + 1) * HD].reshape(2, 128).T)
        m = {"xT": xT, "w_all": w_all, "wout": wo, "bq": bqs, "bk": bks,
             "cos_k": cos_k, "sin_k": sin_k, "Pswap": Pm, "onesblk": onesblk,
             "onesblk8": onesblk8, "ones2blk8": ones2blk8, "sel2": sel2}
        if not shared:
            m["cos_q"] = cos_q
            m["sin_q"] = sin_q
        in_maps.append(m)

    bias_row = (bqkv[2 * DM:] @ Wout + np.asarray(bout, dtype=np.float32)) \
        .astype(np.float32)                                       # [1024]
    return in_maps, shared, float(exp_scale), bias_row


def _install_ntff_shim():
    """Make trace=True usable: this image lacks antenv.axon_hooks; recreate
    it against the baked libaxon_pjrt.so C ABI (no-op if already present)."""
    try:
        from antenv.axon_hooks import get_axon_ntff_profile_hook  # noqa: F401
        return
    except ImportError:
        pass
    try:
        import types, ctypes, contextlib
        import antenv
        lib = ctypes.CDLL("/opt/axon/libaxon_pjrt.so")
        if not hasattr(lib, "axon_start_nrt_profile"):
            raise OSError("no profile symbols")
        lib.axon_start_nrt_profile.argtypes = [ctypes.POINTER(ctypes.c_int64),
                                               ctypes.c_size_t]
        lib.axon_start_nrt_profile.restype = ctypes.c_int64
        lib.axon_stop_nrt_profile.argtypes = [ctypes.c_char_p]
        lib.axon_stop_nrt_profile.restype = ctypes.c_int64

        @contextlib.contextmanager
        def _hook(output_dir, device_ids):
            import jax
            jax.devices()
            if device_ids:
                ids = (ctypes.c_int64 * len(device_ids))(*device_ids)
                rc = lib.axon_start_nrt_profile(ids, len(device_ids))
            else:
                rc = lib.axon_start_nrt_profile(None, 0)
            if rc != 0:
                raise RuntimeError(f"axon_start_nrt_profile rc={rc}")
            try:
                yield
            finally:
                lib.axon_stop_nrt_profile(str(output_dir).encode())

        mod = types.ModuleType("antenv.axon_hooks")
        mod.get_axon_ntff_profile_hook = lambda: _hook
        mod.set_axon_ntff_profile_hook = lambda h: None
        sys.modules["antenv.axon_hooks"] = mod
        antenv.axon_hooks = mod
    except Exception:
        os.environ["BASS_NEVER_TRACE"] = "1"   # degrade: run untraced


def kernel(x, pos, Wqkv, bqkv, Wout, bout, q_scale, k_scale):
    global LAST_RESULTS
    if os.environ.get("BASS_TRACE"):
        _install_ntff_shim()
    in_maps, shared, exp_scale, bias_row = host_prep(
        x, pos, Wqkv, bqkv, Wout, bout, q_scale, k_scale)

    key = (shared, round(exp_scale, 9))
    if key not in _CACHED:
        _CACHED[key] = build_program(exp_scale, shared)
    nc = _CACHED[key]

    res = bass_utils.run_bass_kernel_spmd(
        nc, in_maps, list(range(NC)),
        trace=bool(os.environ.get("BASS_TRACE")))
    LAST_RESULTS = res

    out = np.empty((B, S, DM), dtype=np.float32)
    for b in range(B):
        acc = bias_row[None, :].astype(np.float32).repeat(S, axis=0)
        for g in range(4):
            acc = acc + res.results[b * 4 + g]["outp"].astype(np.float32)
        out[b] = acc
    return out


# revision 27
# speedup vs baseline: 1.1762x; 1.1762x over previous
"""Trainium2 Bass kernel for nn_Attention_32650341384246.

Full attention layer: qkv proj + per-head RMSNorm(q,k) + RoPE + softmax
attention (non-causal) + out proj.  B=2, S=2048, D=1024, H=16, DH=64.

Sharding: 8 cores; core c handles batch c//4, heads [4*(c%4), 4*(c%4)+4)
(data parallel over batch x tensor parallel over heads).  Each core
computes a partial [S, D] output (its heads @ Wout row-slice); the host
sums the 4 partials per batch and adds the (folded) biases.

Device design (per core), v2 (ACT-exp roofline shape):
  - qkv proj emits qT/kT head-major [128 (2 heads x 64), S] (lhsT = W
    slice, rhs = xT slice) and v s-major [s, 4*64].
  - q-path RMSNorm: sumsq via ones-block matmul into a shared [8,512]
    PSUM tile (one Ln+Exp per section), partition-broadcast via
    ones-matmul, u = tt*pb on DVE.
  - k-path RMSNorm is FOLDED INTO THE SOFTMAX EXP: sumsq is computed
    transposed ([s-part, head] via lhsT=sq-slice matmuls into [128,32]
    PSUM), one Ln+Exp gives rsT = exp_scale*rsqrt(ms+eps), and the
    attention exp uses scale=rsT[:, 2*kt+h] (per-partition AP).  The
    k rope output is left unnormalized (rsqrt commutes past rope).
  - RoPE as rot = cosT*u + sinT'*swap(u); swap = adjacent-partition
    permutation matmul (bf16); tables host-built with scales folded.
  - attention: 8 sections (qh, pair, h); per kt: one [128,1024] score
    matmul (double-buffered PSUM scA/scB), one [128,1024] exp, one
    accumulating [65,1024] AV matmul (row 64 = ones = sumexp).  The
    emission is a flat software-pipelined stream: av lags sc/exp by 2
    slots, section normalize (reciprocal + K=1 broadcast matmul +
    vmix mul) is deferred into the next section's slots, so ACT stays
    saturated across section boundaries.
  - out proj: interleaved into the last attention sections (PSUM tag
    shared with the normalize broadcast), copies on DVE, bf16 out.
ACT runs ONLY Exp/Ln (single table set, no reloads).
"""
import sys, os

sys.path.insert(0, "/opt/trn_rl_repo")

import math
import numpy as np
from contextlib import ExitStack

import ml_dtypes
import concourse.bass as bass
import concourse.mybir as mybir
import concourse.tile as tile
from concourse import bacc
from concourse import bass_utils

F32 = mybir.dt.float32
F32R = mybir.dt.float32r
BF16 = mybir.dt.bfloat16
AF = mybir.ActivationFunctionType

B, S, DM, H, DH = 2, 2048, 1024, 16, 64
NC = 8
HPC = H // 4          # 4 heads per core
HD = HPC * DH         # 256
NDT = DM // 128       # 8 model-dim tiles
THETA, EPS = 10000.0, 1e-6

LAST_RESULTS = None   # BassKernelResults of the most recent device run
_CACHED = {}
DEBUG = False         # add debug DMA taps (qt/kt/rsT) for CoreSim bisection


def build_program(exp_scale: float, shared_tables: bool):
    nc = bacc.Bacc("TRN2", target_bir_lowering=False, debug=False)

    xT_d = nc.dram_tensor("xT", [128, NDT, S], BF16, kind="ExternalInput")
    w_d = nc.dram_tensor("w_all", [128, NDT, 3 * HD], BF16, kind="ExternalInput")
    wout_d = nc.dram_tensor("wout", [128, 2, DM], BF16, kind="ExternalInput")
    bq_d = nc.dram_tensor("bq", [128, 2], F32, kind="ExternalInput")
    bk_d = nc.dram_tensor("bk", [128, 2], F32, kind="ExternalInput")
    cosk_d = nc.dram_tensor("cos_k", [128, S], F32, kind="ExternalInput")
    sink_d = nc.dram_tensor("sin_k", [128, S], F32, kind="ExternalInput")
    if not shared_tables:
        cosq_d = nc.dram_tensor("cos_q", [128, S], F32, kind="ExternalInput")
        sinq_d = nc.dram_tensor("sin_q", [128, S], F32, kind="ExternalInput")
    P_d = nc.dram_tensor("Pswap", [128, 128], BF16, kind="ExternalInput")
    ob_d = nc.dram_tensor("onesblk", [128, 2], BF16, kind="ExternalInput")
    ob8_d = nc.dram_tensor("onesblk8", [128, 4, 8], BF16, kind="ExternalInput")
    o2b8_d = nc.dram_tensor("ones2blk8", [8, 4, 128], F32R, kind="ExternalInput")
    sel2_d = nc.dram_tensor("sel2", [1, 2, 128], BF16, kind="ExternalInput")
    out_d = nc.dram_tensor("outp", [S, DM], BF16, kind="ExternalOutput")

    ln_exp_bias = float(math.log(exp_scale)) if exp_scale != 1.0 else 0.0

    with tile.TileContext(nc) as tc, ExitStack() as ctx, \
            nc.allow_low_precision(reason="bf16 matmul inputs"):
        singles = ctx.enter_context(tc.tile_pool(name="singles", bufs=1))
        tmp = ctx.enter_context(tc.tile_pool(name="tmp", bufs=2))
        expp = ctx.enter_context(tc.tile_pool(name="expp", bufs=2))
        outp = ctx.enter_context(tc.tile_pool(name="outp", bufs=4))

        # --- first-needed loads up front; per-dt tiles so Tile's
        # per-tile RAW tracking doesn't serialize readers behind all DMAs ---
        w_dt = [singles.tile([128, 3 * HD], BF16, name=f"w{dt}") for dt in range(NDT)]
        x_dt = [singles.tile([128, S], BF16, name=f"x{dt}") for dt in range(NDT)]
        for dt in range(NDT):
            nc.sync.dma_start(out=w_dt[dt], in_=w_d.ap()[:, dt, :])
            nc.sync.dma_start(out=x_dt[dt], in_=xT_d.ap()[:, dt, :])

        wout = singles.tile([128, 2, DM], BF16)
        nc.sync.dma_start(out=wout, in_=wout_d.ap())
        bq = singles.tile([128, 2], F32)
        nc.sync.dma_start(out=bq, in_=bq_d.ap())
        bk = singles.tile([128, 2], F32)
        nc.sync.dma_start(out=bk, in_=bk_d.ap())
        cos_k = singles.tile([128, S], F32)
        nc.sync.dma_start(out=cos_k, in_=cosk_d.ap())
        sin_k = singles.tile([128, S], F32)
        nc.sync.dma_start(out=sin_k, in_=sink_d.ap())
        if shared_tables:
            cos_q, sin_q = cos_k, sin_k
        else:
            cos_q = singles.tile([128, S], F32)
            nc.sync.dma_start(out=cos_q, in_=cosq_d.ap())
            sin_q = singles.tile([128, S], F32)
            nc.sync.dma_start(out=sin_q, in_=sinq_d.ap())
        Pm = singles.tile([128, 128], BF16)
        nc.sync.dma_start(out=Pm, in_=P_d.ap())
        onesblk = singles.tile([128, 2], BF16)
        nc.sync.dma_start(out=onesblk, in_=ob_d.ap())
        onesblk8 = singles.tile([128, 4, 8], BF16)
        nc.sync.dma_start(out=onesblk8, in_=ob8_d.ap())
        ones2blk8 = singles.tile([8, 4, 128], F32R)
        nc.sync.dma_start(out=ones2blk8, in_=o2b8_d.ap())
        sel2 = singles.tile([1, 2, 128], BF16)
        nc.sync.dma_start(out=sel2, in_=sel2_d.ap())
        eps8 = singles.tile([8, 1], F32)
        nc.vector.memset(eps8, EPS)
        eps128 = singles.tile([128, 1], F32)
        nc.vector.memset(eps128, EPS)
        lnb128 = singles.tile([128, 1], F32)
        nc.vector.memset(lnb128, ln_exp_bias)

        # head-major roped q: [128 (2 heads x 64 dims), 2048 s] per pair
        qt = [singles.tile([128, S], BF16, name=f"qt{t}") for t in range(2)]
        # k zero-padded per head: kth[t][:, h, :] has head-h dims in rows
        # 64h..64h+63, zeros elsewhere -> K=128 score matmuls, no PE
        # row-tiling mode switches against the K=128 AV matmuls.
        kth = [singles.tile([128, 2, S], BF16, name=f"kth{t}") for t in range(2)]
        for t in range(2):
            nc.gpsimd.memset(kth[t][:, :, :], 0.0)
        # rsT[t][:, 2*kt+h] = exp_scale * rsqrt(mean k^2 + eps) per k-pos
        rsT = [singles.tile([128, 32], F32, name=f"rsT{t}") for t in range(2)]
        vhat = [singles.tile([128, 4, HPC, 65], BF16, name=f"vhat{sc}")
                for sc in range(4)]
        for sc in range(4):
            nc.vector.memset(vhat[sc][:, :, :, 64:65], 1.0)
        vmix = [[singles.tile([128, 1024], BF16, name=f"vmix{t}_{qh}")
                 for qh in range(2)] for t in range(2)]

        # ---------------- phase 1: qkv + rmsnorm + rope ----------------
        with tc.tile_pool(name="ps1", bufs=1, space="PSUM") as ps1:
            def emit_A(which, t):
                """proj + bias + square + sumsq (+ k: transposed rsqrt)."""
                off = 0 if which == "q" else HD
                bias = bq if which == "q" else bk
                tts = []
                if which == "q":
                    pss = ps1.tile([8, 512], F32, tag="pss",
                                   name=f"pss{which}{t}")
                else:
                    rstp = ps1.tile([128, 32], F32, tag="rstp",
                                    name=f"rstp{t}")
                for sc in range(4):
                    s0 = sc * 512
                    pq = ps1.tile([128, 512], F32, tag="pq", bufs=2,
                                  name=f"pq{which}{t}_{sc}")
                    for dt in range(NDT):
                        nc.tensor.matmul(
                            pq[:, :],
                            w_dt[dt][:, off + t * 128: off + (t + 1) * 128],
                            x_dt[dt][:, s0:s0 + 512],
                            start=(dt == 0), stop=(dt == NDT - 1))
                    tt = tmp.tile([128, 512], BF16, tag="tt", bufs=10,
                                  name=f"tt{which}{t}_{sc}")
                    nc.scalar.activation(tt[:, :], pq[:, :], AF.Identity,
                                         bias=bias[:, t:t + 1], scale=1.0)
                    tts.append(tt)
                    sq = tmp.tile([128, 512], BF16, tag="sq", bufs=3,
                                  name=f"sq{which}{t}_{sc}")
                    nc.scalar.activation(sq[:, :], pq[:, :], AF.Square,
                                         bias=bias[:, t:t + 1], scale=1.0)
                    if which == "q":
                        nc.tensor.matmul(pss[:, :], onesblk8[:, sc, :],
                                         sq[:, :],
                                         start=(sc == 0), stop=(sc == 3))
                    else:
                        for qtr in range(4):
                            kt = 4 * sc + qtr
                            nc.tensor.matmul(
                                rstp[:, 2 * kt:2 * kt + 2],
                                sq[:, qtr * 128:(qtr + 1) * 128],
                                onesblk[:, :],
                                start=True, stop=True)
                if which == "q":
                    lns = tmp.tile([8, 512], F32, tag="lns",
                                   name=f"lns{which}{t}")
                    nc.scalar.activation(lns[:, :], pss[:, :], AF.Ln,
                                         bias=eps8[:, :], scale=1.0 / DH)
                    rs = tmp.tile([8, 512], F32R, tag="rs",
                                  name=f"rs{which}{t}")
                    nc.scalar.activation(rs[:, :], lns[:, :], AF.Exp,
                                         scale=-0.5)
                    return tts, rs
                else:
                    lnt = tmp.tile([128, 32], F32, tag="lnt",
                                   name=f"lnt{t}")
                    nc.scalar.activation(lnt[:, :], rstp[:, :], AF.Ln,
                                         bias=eps128[:, :], scale=1.0 / DH)
                    nc.scalar.activation(rsT[t][:, :], lnt[:, :], AF.Exp,
                                         scale=-0.5, bias=lnb128[:, :])
                    return tts, None

            def emit_B(which, t, tts, rs):
                """(q: normalize) + rope -> qt/kth tiles."""
                cosT = cos_q if which == "q" else cos_k
                sinT = sin_q if which == "q" else sin_k
                for sc in range(4):
                    s0 = sc * 512
                    if which == "q":
                        pb = ps1.tile([128, 512], F32, tag="pb",
                                      name=f"pb{which}{t}_{sc}")
                        nc.tensor.matmul(pb[:, :], ones2blk8[:, sc, :],
                                         rs[:, :], start=True, stop=True)
                        u = tmp.tile([128, 512], BF16, tag="u", bufs=3,
                                     name=f"u{which}{t}_{sc}")
                        nc.vector.tensor_mul(u[:, :], tts[sc][:, :], pb[:, :])
                    else:
                        u = tts[sc]
                    psw = ps1.tile([128, 512], F32, tag="psw",
                                   name=f"psw{which}{t}_{sc}")
                    nc.tensor.matmul(psw[:, :], Pm[:, :], u[:, :],
                                     start=True, stop=True)
                    t1 = tmp.tile([128, 512], F32, tag="t1", bufs=3,
                                  name=f"t1{which}{t}_{sc}")
                    nc.vector.tensor_mul(t1[:, :], u[:, :],
                                         cosT[:, s0:s0 + 512])
                    t2 = tmp.tile([128, 512], F32, tag="t2", bufs=3,
                                  name=f"t2{which}{t}_{sc}")
                    nc.vector.tensor_mul(t2[:, :], psw[:, :],
                                         sinT[:, s0:s0 + 512])
                    if which == "q":
                        nc.vector.tensor_add(qt[t][:, s0:s0 + 512],
                                             t1[:, :], t2[:, :])
                    else:
                        for hh in range(2):
                            nc.vector.tensor_add(
                                kth[t][hh * 64:(hh + 1) * 64, hh,
                                       s0:s0 + 512],
                                t1[hh * 64:(hh + 1) * 64, :],
                                t2[hh * 64:(hh + 1) * 64, :])

            def emit_V():
                for sc in range(4):
                    for st in range(4):
                        pv = ps1.tile([128, HD], F32, tag="pv", bufs=2,
                                      name=f"pv{sc}_{st}")
                        for dt in range(NDT):
                            nc.tensor.matmul(
                                pv[:, :],
                                x_dt[dt][:, sc * 512 + st * 128:
                                         sc * 512 + (st + 1) * 128],
                                w_dt[dt][:, 2 * HD:3 * HD],
                                start=(dt == 0), stop=(dt == NDT - 1))
                        nc.scalar.copy(
                            vhat[sc][:, st, :, 0:64],
                            pv[:, :].rearrange("p (h d) -> p h d", h=HPC))

            # A/B software pipeline: B(x) consumes rsqrt computed during
            # the next A's matmuls, so PE never stalls on ACT.
            a_k0 = emit_A("k", 0)
            a_q0 = emit_A("q", 0)
            emit_B("k", 0, *a_k0)
            a_k1 = emit_A("k", 1)
            emit_B("q", 0, *a_q0)
            a_q1 = emit_A("q", 1)
            emit_B("k", 1, *a_k1)
            emit_B("q", 1, *a_q1)
            emit_V()

        if DEBUG:
            for t in range(2):
                for hh in range(2):
                    d = nc.dram_tensor(f"dbg_kth{t}{hh}", [128, S], BF16,
                                       kind="ExternalOutput")
                    nc.sync.dma_start(out=d.ap(), in_=kth[t][:, hh, :])
                d = nc.dram_tensor(f"dbg_qt{t}", [128, S], BF16,
                                   kind="ExternalOutput")
                nc.sync.dma_start(out=d.ap(), in_=qt[t][:, :])
                d = nc.dram_tensor(f"dbg_rsT{t}", [128, 32], F32,
                                   kind="ExternalOutput")
                nc.sync.dma_start(out=d.ap(), in_=rsT[t][:, :])

        # ---------------- phase 2: attention (+ interleaved out proj) ---
        with tc.tile_pool(name="ps2", bufs=1, space="PSUM") as ps2:
            from collections import deque
            avq = deque()       # pending av / normalize emitters
            po_units = deque()  # pending out-proj emitters

            def make_po(st, on_act=False):
                def emit():
                    qh = st // 8
                    po = ps2.tile([128, 1024], F32, tag="nb",
                                  name=f"po{st}")
                    for qc in range(2):
                        for t in range(2):
                            nc.tensor.matmul(
                                po[:, qc * 512:(qc + 1) * 512],
                                vmix[t][qh][:, (st % 8) * 128:(st % 8 + 1) * 128],
                                wout[:, t, qc * 512:(qc + 1) * 512],
                                start=(t == 0), stop=(t == 1))
                    o = outp.tile([128, 1024], BF16, tag="o", name=f"o{st}")
                    if on_act:
                        nc.scalar.copy(o[:, :], po[:, :])
                    else:
                        nc.vector.tensor_copy(o[:, :], po[:, :])
                    nc.sync.dma_start(
                        out=out_d.ap()[st * 128:(st + 1) * 128, :],
                        in_=o[:, :])
                return emit

            sections = [(pair, h, qh) for qh in range(2)
                        for pair in range(2) for h in range(2)]
            for si, (pair, h, qh) in enumerate(sections):
                q0 = qh * 1024
                head = 2 * pair + h
                ps_sc = [ps2.tile([128, 1024], F32, tag=f"sc{ab}",
                                  name=f"sc{si}_{ab}") for ab in range(2)]
                pav = ps2.tile([65, 1024], F32, tag="av", name=f"av{si}")
                es = {}
                for kt in range(16):
                    pssc = ps_sc[kt % 2]
                    for qc in range(2):
                        nc.tensor.matmul(
                            pssc[:, qc * 512:(qc + 1) * 512],
                            kth[pair][:, h, kt * 128:(kt + 1) * 128],
                            qt[pair][:, q0 + qc * 512:q0 + (qc + 1) * 512],
                            start=True, stop=True)
                    e = expp.tile([128, 1024], BF16, tag=f"e{kt % 4}",
                                  name=f"e{si}_{kt}")
                    nc.scalar.activation(
                        e[:, :], pssc[:, :], AF.Exp,
                        scale=rsT[pair][:, 2 * kt + h:2 * kt + h + 1])
                    es[kt] = e
                    # drain one pending unit (av lags by ~2 slots)
                    if kt == 6 and po_units:
                        po_units.popleft()()
                    if avq:
                        avq.popleft()()
                        if kt == 3 and avq:   # catch up after normalize slot
                            avq.popleft()()
                    if kt == 11 and po_units:
                        po_units.popleft()()

                    def make_av(kt, pav=pav, es=es, head=head, si=si):
                        def emit():
                            ek = es[kt]
                            for qc in range(2):
                                nc.tensor.matmul(
                                    pav[:, qc * 512:(qc + 1) * 512],
                                    vhat[kt // 4][:, kt % 4, head, :],
                                    ek[:, qc * 512:(qc + 1) * 512],
                                    start=(kt == 0), stop=(kt == 15),
                                    skip_group_check=True)
                        return emit
                    if kt >= 2:
                        make_av(kt - 2)()
                # leftovers: av(14), av(15), then normalize
                avq.append(make_av(14))
                avq.append(make_av(15))

                def make_norm(pair=pair, h=h, qh=qh, pav=pav, si=si):
                    def emit():
                        se = tmp.tile([1, 1024], F32, tag="se",
                                      name=f"se{si}")
                        nc.vector.tensor_copy(se[:, :], pav[64:65, :])
                        rc = tmp.tile([1, 1024], BF16, tag="rc",
                                      name=f"rc{si}")
                        from concourse.dve_ops import (
                            RECIP_APPROX_FAST_CONSTS, RECIPROCAL_APPROX_FAST)
                        _c = RECIP_APPROX_FAST_CONSTS
                        nc.vector._custom_dve(RECIPROCAL_APPROX_FAST,
                                              out=rc[:, :],
                                              in0=se[:, :],
                                              s0=_c["s0"], s1=_c["s1"],
                                              imm2=_c["imm2"])
                        nb = ps2.tile([128, 1024], F32, tag="nb",
                                      name=f"nb{si}")
                        for qc in range(2):
                            nc.tensor.matmul(nb[:, qc * 512:(qc + 1) * 512],
                                             sel2[:, h, :],
                                             rc[:, qc * 512:(qc + 1) * 512],
                                             start=True, stop=True)
                        avs = tmp.tile([64, 1024], F32, tag="avs",
                                       name=f"avs{si}")
                        nc.vector.tensor_copy(avs[:, :], pav[0:64, :])
                        nc.vector.tensor_mul(
                            vmix[pair][qh][h * 64:(h + 1) * 64, :],
                            avs[:, :], nb[h * 64:(h + 1) * 64, :])
                    return emit
                avq.append(make_norm())
                if si == 3:      # vmix[*][qh0] complete after section 3
                    for st in range(8):
                        po_units.append(make_po(st))

            # tail: drain remaining av/normalize, then out proj qh1
            while avq:
                avq.popleft()()
            for st in range(8, 16):
                po_units.append(make_po(st, on_act=(st % 2 == 0)))
            while po_units:
                po_units.popleft()()

            if DEBUG:
                for t in range(2):
                    for qh in range(2):
                        d = nc.dram_tensor(f"dbg_vmix{t}{qh}", [128, 1024],
                                           BF16, kind="ExternalOutput")
                        nc.sync.dma_start(out=d.ap(), in_=vmix[t][qh][:, :])

    nc.compile()
    return nc


def host_prep(x, pos, Wqkv, bqkv, Wout, bout, q_scale, k_scale):
    """Build per-core input maps + shared-table decision."""
    x = np.asarray(x, dtype=np.float32)
    pos = np.asarray(pos, dtype=np.float32).reshape(-1)
    Wqkv = np.asarray(Wqkv, dtype=np.float32)
    bqkv = np.asarray(bqkv, dtype=np.float32)
    Wout = np.asarray(Wout, dtype=np.float32)
    q_scale = np.asarray(q_scale, dtype=np.float32)
    k_scale = np.asarray(k_scale, dtype=np.float32)

    shared = bool(np.array_equal(q_scale, k_scale))
    exp_scale = (1.0 / np.sqrt(DH)) if shared else 1.0

    # rope base tables [128, S]
    i_of_p = (np.arange(128) % 64) // 2            # pair index
    sign = np.where(np.arange(128) % 2 == 0, 1.0, -1.0)
    omega = THETA ** (-np.arange(0, DH, 2, dtype=np.float64) / DH)  # [32]
    ang = pos[None, :].astype(np.float64) * omega[:, None]          # [32, S]
    cosb = np.cos(ang)[i_of_p, :]                  # [128, S]
    sinb = np.sin(ang)[i_of_p, :] * sign[:, None]

    def tables(scale_vec, extra):
        sv = np.tile(scale_vec, 2)                 # [128]
        svx = np.tile(scale_vec[np.arange(64) ^ 1], 2)
        cosT = (cosb * sv[:, None] * extra).astype(np.float32)
        sinT = (sinb * svx[:, None] * extra).astype(np.float32)
        return np.ascontiguousarray(cosT), np.ascontiguousarray(sinT)

    cos_k, sin_k = tables(k_scale, 1.0)
    if not shared:
        cos_q, sin_q = tables(q_scale, 1.0 / np.sqrt(DH))

    bf = ml_dtypes.bfloat16
    Pm = np.zeros((128, 128), dtype=bf)
    Pm[np.arange(128), np.arange(128) ^ 1] = 1.0
    onesblk = np.zeros((128, 2), dtype=bf)
    onesblk[0:64, 0] = 1.0
    onesblk[64:128, 1] = 1.0
    # q-path sumsq gather: [8,512] rows (2*sc, 2*sc+1) = head halves
    onesblk8 = np.zeros((128, 4, 8), dtype=bf)
    ones2blk8 = np.zeros((8, 4, 128), dtype=np.float32)
    for sc in range(4):
        onesblk8[0:64, sc, 2 * sc] = 1.0
        onesblk8[64:128, sc, 2 * sc + 1] = 1.0
        ones2blk8[2 * sc, sc, 0:64] = 1.0
        ones2blk8[2 * sc + 1, sc, 64:128] = 1.0
    sel2 = np.zeros((1, 2, 128), dtype=bf)
    sel2[0, 0, 0:64] = 1.0
    sel2[0, 1, 64:128] = 1.0

    in_maps = []
    for c in range(NC):
        b, g = c // 4, c % 4
        xT = np.ascontiguousarray(
            x[b].T.reshape(NDT, 128, S).transpose(1, 0, 2)).astype(bf)
        wq = Wqkv[:, g * HD:(g + 1) * HD]
        wk = Wqkv[:, DM + g * HD: DM + (g + 1) * HD]
        wv = Wqkv[:, 2 * DM + g * HD: 2 * DM + (g + 1) * HD]
        w_all = np.ascontiguousarray(
            np.concatenate([wq, wk, wv], axis=1)
            .reshape(NDT, 128, 3 * HD).transpose(1, 0, 2)).astype(bf)
        wo = np.ascontiguousarray(
            Wout[g * HD:(g + 1) * HD, :]
            .reshape(2, 128, DM).transpose(1, 0, 2)).astype(bf)
        bqs = np.ascontiguousarray(
            bqkv[g * HD:(g + 1) * HD].reshape(2, 128).T)         # [128, 2]
        bks = np.ascontiguousarray(
            bqkv[DM + g * HD: DM + (g + 1) * HD].reshape(2, 128).T)
        m = {"xT": xT, "w_all": w_all, "wout": wo, "bq": bqs, "bk": bks,
             "cos_k": cos_k, "sin_k": sin_k, "Pswap": Pm, "onesblk": onesblk,
             "onesblk8": onesblk8, "ones2blk8": ones2blk8, "sel2": sel2}
        if not shared:
            m["cos_q"] = cos_q
            m["sin_q"] = sin_q
        in_maps.append(m)

    bias_row = (bqkv[2 * DM:] @ Wout + np.asarray(bout, dtype=np.float32)) \
        .astype(np.float32)                                       # [1024]
    return in_maps, shared, float(exp_scale), bias_row


def _install_ntff_shim():
    """Make trace=True usable: this image lacks antenv.axon_hooks; recreate
    it against the baked libaxon_pjrt.so C ABI (no-op if already present)."""
    try:
        from antenv.axon_hooks import get_axon_ntff_profile_hook  # noqa: F401
        return
    except ImportError:
        pass
    try:
        import types, ctypes, contextlib
        import antenv
        lib = ctypes.CDLL("/opt/axon/libaxon_pjrt.so")
        if not hasattr(lib, "axon_start_nrt_profile"):
            raise OSError("no profile symbols")
        lib.axon_start_nrt_profile.argtypes = [ctypes.POINTER(ctypes.c_int64),
                                               ctypes.c_size_t]
        lib.axon_start_nrt_profile.restype = ctypes.c_int64
        lib.axon_stop_nrt_profile.argtypes = [ctypes.c_char_p]
        lib.axon_stop_nrt_profile.restype = ctypes.c_int64

        @contextlib.contextmanager
        def _hook(output_dir, device_ids):
            import jax
            jax.devices()
            if device_ids:
                ids = (ctypes.c_int64 * len(device_ids))(*device_ids)
                rc = lib.axon_start_nrt_profile(ids, len(device_ids))
            else:
                rc = lib.axon_start_nrt_profile(None, 0)
            if rc != 0:
                raise RuntimeError(f"axon_start_nrt_profile rc={rc}")
            try:
                yield
            finally:
                lib.axon_stop_nrt_profile(str(output_dir).encode())

        mod = types.ModuleType("antenv.axon_hooks")
        mod.get_axon_ntff_profile_hook = lambda: _hook
        mod.set_axon_ntff_profile_hook = lambda h: None
        sys.modules["antenv.axon_hooks"] = mod
        antenv.axon_hooks = mod
    except Exception:
        os.environ["BASS_NEVER_TRACE"] = "1"   # degrade: run untraced


def kernel(x, pos, Wqkv, bqkv, Wout, bout, q_scale, k_scale):
    global LAST_RESULTS
    if os.environ.get("BASS_TRACE"):
        _install_ntff_shim()
    in_maps, shared, exp_scale, bias_row = host_prep(
        x, pos, Wqkv, bqkv, Wout, bout, q_scale, k_scale)

    key = (shared, round(exp_scale, 9))
    if key not in _CACHED:
        _CACHED[key] = build_program(exp_scale, shared)
    nc = _CACHED[key]

    res = bass_utils.run_bass_kernel_spmd(
        nc, in_maps, list(range(NC)),
        trace=bool(os.environ.get("BASS_TRACE")))
    LAST_RESULTS = res

    out = np.empty((B, S, DM), dtype=np.float32)
    for b in range(B):
        acc = bias_row[None, :].astype(np.float32).repeat(S, axis=0)
        for g in range(4):
            acc = acc + res.results[b * 4 + g]["outp"].astype(np.float32)
        out[b] = acc
    return out


# revision 28
# speedup vs baseline: 1.2661x; 1.0764x over previous
"""Trainium2 Bass kernel for nn_Attention_32650341384246.

Full attention layer: qkv proj + per-head RMSNorm(q,k) + RoPE + softmax
attention (non-causal) + out proj.  B=2, S=2048, D=1024, H=16, DH=64.

Sharding: 8 cores; core c handles batch c//4, heads [4*(c%4), 4*(c%4)+4)
(data parallel over batch x tensor parallel over heads).  Each core
computes a partial [S, D] output (its heads @ Wout row-slice); the host
sums the 4 partials per batch and adds the (folded) biases.

Device design (per core), v2 (ACT-exp roofline shape):
  - qkv proj emits qT/kT head-major [128 (2 heads x 64), S] (lhsT = W
    slice, rhs = xT slice) and v s-major [s, 4*64].
  - q-path RMSNorm: sumsq via ones-block matmul into a shared [8,512]
    PSUM tile (one Ln+Exp per section), partition-broadcast via
    ones-matmul, u = tt*pb on DVE.
  - k-path RMSNorm is FOLDED INTO THE SOFTMAX EXP: sumsq is computed
    transposed ([s-part, head] via lhsT=sq-slice matmuls into [128,32]
    PSUM), one Ln+Exp gives rsT = exp_scale*rsqrt(ms+eps), and the
    attention exp uses scale=rsT[:, 2*kt+h] (per-partition AP).  The
    k rope output is left unnormalized (rsqrt commutes past rope).
  - RoPE as rot = cosT*u + sinT'*swap(u); swap = adjacent-partition
    permutation matmul (bf16); tables host-built with scales folded.
  - attention: 8 sections (qh, pair, h); per kt: one [128,1024] score
    matmul (double-buffered PSUM scA/scB), one [128,1024] exp, one
    accumulating [65,1024] AV matmul (row 64 = ones = sumexp).  The
    emission is a flat software-pipelined stream: av lags sc/exp by 2
    slots, section normalize (reciprocal + K=1 broadcast matmul +
    vmix mul) is deferred into the next section's slots, so ACT stays
    saturated across section boundaries.
  - out proj: interleaved into the last attention sections (PSUM tag
    shared with the normalize broadcast), copies on DVE, bf16 out.
ACT runs ONLY Exp/Ln (single table set, no reloads).
"""
import sys, os

sys.path.insert(0, "/opt/trn_rl_repo")

import math
import numpy as np
from contextlib import ExitStack

import ml_dtypes
import concourse.bass as bass
import concourse.mybir as mybir
import concourse.tile as tile
from concourse import bacc
from concourse import bass_utils

F32 = mybir.dt.float32
F32R = mybir.dt.float32r
BF16 = mybir.dt.bfloat16
AF = mybir.ActivationFunctionType

B, S, DM, H, DH = 2, 2048, 1024, 16, 64
NC = 8
HPC = H // 4          # 4 heads per core
HD = HPC * DH         # 256
NDT = DM // 128       # 8 model-dim tiles
THETA, EPS = 10000.0, 1e-6

LAST_RESULTS = None   # BassKernelResults of the most recent device run
_CACHED = {}
DEBUG = False         # add debug DMA taps (qt/kt/rsT) for CoreSim bisection


def build_program(exp_scale: float, shared_tables: bool):
    nc = bacc.Bacc("TRN2", target_bir_lowering=False, debug=False)

    xT_d = nc.dram_tensor("xT", [128, NDT, S], BF16, kind="ExternalInput")
    w_d = nc.dram_tensor("w_all", [128, NDT, 3 * HD], BF16, kind="ExternalInput")
    wout_d = nc.dram_tensor("wout", [128, 2, DM], BF16, kind="ExternalInput")
    bq_d = nc.dram_tensor("bq", [128, 2], F32, kind="ExternalInput")
    bk_d = nc.dram_tensor("bk", [128, 2], F32, kind="ExternalInput")
    cosk_d = nc.dram_tensor("cos_k", [128, S], F32, kind="ExternalInput")
    sink_d = nc.dram_tensor("sin_k", [128, S], F32, kind="ExternalInput")
    if not shared_tables:
        cosq_d = nc.dram_tensor("cos_q", [128, S], F32, kind="ExternalInput")
        sinq_d = nc.dram_tensor("sin_q", [128, S], F32, kind="ExternalInput")
    P_d = nc.dram_tensor("Pswap", [128, 128], BF16, kind="ExternalInput")
    ob_d = nc.dram_tensor("onesblk", [128, 2], BF16, kind="ExternalInput")
    ob8_d = nc.dram_tensor("onesblk8", [128, 4, 8], BF16, kind="ExternalInput")
    o2b8_d = nc.dram_tensor("ones2blk8", [8, 4, 128], F32R, kind="ExternalInput")
    sel2_d = nc.dram_tensor("sel2", [1, 2, 128], BF16, kind="ExternalInput")
    out_d = nc.dram_tensor("outp", [S, DM], BF16, kind="ExternalOutput")

    ln_exp_bias = float(math.log(exp_scale)) if exp_scale != 1.0 else 0.0

    with tile.TileContext(nc) as tc, ExitStack() as ctx, \
            nc.allow_low_precision(reason="bf16 matmul inputs"):
        singles = ctx.enter_context(tc.tile_pool(name="singles", bufs=1))
        tmp = ctx.enter_context(tc.tile_pool(name="tmp", bufs=2))
        expp = ctx.enter_context(tc.tile_pool(name="expp", bufs=2))
        outp = ctx.enter_context(tc.tile_pool(name="outp", bufs=4))

        # --- first-needed loads up front; per-dt tiles so Tile's
        # per-tile RAW tracking doesn't serialize readers behind all DMAs ---
        w_dt = [singles.tile([128, 3 * HD], BF16, name=f"w{dt}") for dt in range(NDT)]
        x_dt = [singles.tile([128, S], BF16, name=f"x{dt}") for dt in range(NDT)]
        for dt in range(NDT):
            nc.sync.dma_start(out=w_dt[dt], in_=w_d.ap()[:, dt, :])
            nc.sync.dma_start(out=x_dt[dt], in_=xT_d.ap()[:, dt, :])

        wout = singles.tile([128, 2, DM], BF16)
        nc.sync.dma_start(out=wout, in_=wout_d.ap())
        bq = singles.tile([128, 2], F32)
        nc.sync.dma_start(out=bq, in_=bq_d.ap())
        bk = singles.tile([128, 2], F32)
        nc.sync.dma_start(out=bk, in_=bk_d.ap())
        cos_k = singles.tile([128, S], F32)
        nc.sync.dma_start(out=cos_k, in_=cosk_d.ap())
        sin_k = singles.tile([128, S], F32)
        nc.sync.dma_start(out=sin_k, in_=sink_d.ap())
        if shared_tables:
            cos_q, sin_q = cos_k, sin_k
        else:
            cos_q = singles.tile([128, S], F32)
            nc.sync.dma_start(out=cos_q, in_=cosq_d.ap())
            sin_q = singles.tile([128, S], F32)
            nc.sync.dma_start(out=sin_q, in_=sinq_d.ap())
        Pm = singles.tile([128, 128], BF16)
        nc.sync.dma_start(out=Pm, in_=P_d.ap())
        onesblk = singles.tile([128, 2], BF16)
        nc.sync.dma_start(out=onesblk, in_=ob_d.ap())
        onesblk8 = singles.tile([128, 4, 8], BF16)
        nc.sync.dma_start(out=onesblk8, in_=ob8_d.ap())
        ones2blk8 = singles.tile([8, 4, 128], F32R)
        nc.sync.dma_start(out=ones2blk8, in_=o2b8_d.ap())
        sel2 = singles.tile([1, 2, 128], BF16)
        nc.sync.dma_start(out=sel2, in_=sel2_d.ap())
        eps8 = singles.tile([8, 1], F32)
        nc.vector.memset(eps8, EPS)
        eps128 = singles.tile([128, 1], F32)
        nc.vector.memset(eps128, EPS)
        lnb128 = singles.tile([128, 1], F32)
        nc.vector.memset(lnb128, ln_exp_bias)

        # head-major roped q: [128 (2 heads x 64 dims), 2048 s] per pair
        qt = [singles.tile([128, S], BF16, name=f"qt{t}") for t in range(2)]
        # k zero-padded per head: kth[t][:, h, :] has head-h dims in rows
        # 64h..64h+63, zeros elsewhere -> K=128 score matmuls, no PE
        # row-tiling mode switches against the K=128 AV matmuls.
        kth = [singles.tile([128, 2, S], BF16, name=f"kth{t}") for t in range(2)]
        for t in range(2):
            nc.gpsimd.memset(kth[t][:, :, :], 0.0)
        # rsT[t][:, 2*kt+h] = exp_scale * rsqrt(mean k^2 + eps) per k-pos
        rsT = [singles.tile([128, 32], F32, name=f"rsT{t}") for t in range(2)]
        vhat = [singles.tile([128, 4, HPC, 65], BF16, name=f"vhat{sc}")
                for sc in range(4)]
        for sc in range(4):
            nc.vector.memset(vhat[sc][:, :, :, 64:65], 1.0)
        vmix = [[singles.tile([128, 1024], BF16, name=f"vmix{t}_{qh}")
                 for qh in range(2)] for t in range(2)]

        # ---------------- phase 1: qkv + rmsnorm + rope ----------------
        with tc.tile_pool(name="ps1", bufs=1, space="PSUM") as ps1:
            def emit_A(which, t):
                """proj + bias + square + sumsq (+ k: transposed rsqrt)."""
                off = 0 if which == "q" else HD
                bias = bq if which == "q" else bk
                tts = []
                if which == "q":
                    pss = ps1.tile([8, 512], F32, tag="pss",
                                   name=f"pss{which}{t}")
                else:
                    rstp = ps1.tile([128, 32], F32, tag="rstp",
                                    name=f"rstp{t}")
                for sc in range(4):
                    s0 = sc * 512
                    pq = ps1.tile([128, 512], F32, tag="pq", bufs=2,
                                  name=f"pq{which}{t}_{sc}")
                    for dt in range(NDT):
                        nc.tensor.matmul(
                            pq[:, :],
                            w_dt[dt][:, off + t * 128: off + (t + 1) * 128],
                            x_dt[dt][:, s0:s0 + 512],
                            start=(dt == 0), stop=(dt == NDT - 1))
                    tt = tmp.tile([128, 512], BF16, tag="tt", bufs=10,
                                  name=f"tt{which}{t}_{sc}")
                    nc.scalar.activation(tt[:, :], pq[:, :], AF.Identity,
                                         bias=bias[:, t:t + 1], scale=1.0)
                    tts.append(tt)
                    sq = tmp.tile([128, 512], BF16, tag="sq", bufs=3,
                                  name=f"sq{which}{t}_{sc}")
                    nc.scalar.activation(sq[:, :], pq[:, :], AF.Square,
                                         bias=bias[:, t:t + 1], scale=1.0)
                    if which == "q":
                        nc.tensor.matmul(pss[:, :], onesblk8[:, sc, :],
                                         sq[:, :],
                                         start=(sc == 0), stop=(sc == 3))
                    else:
                        for qtr in range(4):
                            kt = 4 * sc + qtr
                            nc.tensor.matmul(
                                rstp[:, 2 * kt:2 * kt + 2],
                                sq[:, qtr * 128:(qtr + 1) * 128],
                                onesblk[:, :],
                                start=True, stop=True)
                if which == "q":
                    lns = tmp.tile([8, 512], F32, tag="lns",
                                   name=f"lns{which}{t}")
                    nc.scalar.activation(lns[:, :], pss[:, :], AF.Ln,
                                         bias=eps8[:, :], scale=1.0 / DH)
                    rs = tmp.tile([8, 512], F32R, tag="rs",
                                  name=f"rs{which}{t}")
                    nc.scalar.activation(rs[:, :], lns[:, :], AF.Exp,
                                         scale=-0.5)
                    return tts, rs
                else:
                    lnt = tmp.tile([128, 32], F32, tag="lnt",
                                   name=f"lnt{t}")
                    nc.scalar.activation(lnt[:, :], rstp[:, :], AF.Ln,
                                         bias=eps128[:, :], scale=1.0 / DH)
                    nc.scalar.activation(rsT[t][:, :], lnt[:, :], AF.Exp,
                                         scale=-0.5, bias=lnb128[:, :])
                    return tts, None

            def emit_B(which, t, tts, rs):
                """(q: normalize) + rope -> qt/kth tiles."""
                cosT = cos_q if which == "q" else cos_k
                sinT = sin_q if which == "q" else sin_k
                for sc in range(4):
                    s0 = sc * 512
                    if which == "q":
                        pb = ps1.tile([128, 512], F32, tag="pb",
                                      name=f"pb{which}{t}_{sc}")
                        nc.tensor.matmul(pb[:, :], ones2blk8[:, sc, :],
                                         rs[:, :], start=True, stop=True)
                        u = tmp.tile([128, 512], BF16, tag="u", bufs=3,
                                     name=f"u{which}{t}_{sc}")
                        nc.vector.tensor_mul(u[:, :], tts[sc][:, :], pb[:, :])
                    else:
                        u = tts[sc]
                    psw = ps1.tile([128, 512], F32, tag="psw",
                                   name=f"psw{which}{t}_{sc}")
                    nc.tensor.matmul(psw[:, :], Pm[:, :], u[:, :],
                                     start=True, stop=True)
                    t1 = tmp.tile([128, 512], F32, tag="t1", bufs=3,
                                  name=f"t1{which}{t}_{sc}")
                    nc.gpsimd.tensor_mul(t1[:, :], u[:, :],
                                         cosT[:, s0:s0 + 512])
                    t2 = tmp.tile([128, 512], F32, tag="t2", bufs=3,
                                  name=f"t2{which}{t}_{sc}")
                    nc.vector.tensor_mul(t2[:, :], psw[:, :],
                                         sinT[:, s0:s0 + 512])
                    if which == "q":
                        nc.gpsimd.tensor_add(qt[t][:, s0:s0 + 512],
                                             t1[:, :], t2[:, :])
                    else:
                        for hh in range(2):
                            nc.gpsimd.tensor_add(
                                kth[t][hh * 64:(hh + 1) * 64, hh,
                                       s0:s0 + 512],
                                t1[hh * 64:(hh + 1) * 64, :],
                                t2[hh * 64:(hh + 1) * 64, :])

            def emit_V():
                for sc in range(4):
                    for st in range(4):
                        pv = ps1.tile([128, HD], F32, tag="pv", bufs=2,
                                      name=f"pv{sc}_{st}")
                        for dt in range(NDT):
                            nc.tensor.matmul(
                                pv[:, :],
                                x_dt[dt][:, sc * 512 + st * 128:
                                         sc * 512 + (st + 1) * 128],
                                w_dt[dt][:, 2 * HD:3 * HD],
                                start=(dt == 0), stop=(dt == NDT - 1))
                        nc.scalar.copy(
                            vhat[sc][:, st, :, 0:64],
                            pv[:, :].rearrange("p (h d) -> p h d", h=HPC))

            # A/B software pipeline: B(x) consumes rsqrt computed during
            # the next A's matmuls, so PE never stalls on ACT.
            a_k0 = emit_A("k", 0)
            a_q0 = emit_A("q", 0)
            emit_B("k", 0, *a_k0)
            a_k1 = emit_A("k", 1)
            emit_B("q", 0, *a_q0)
            a_q1 = emit_A("q", 1)
            emit_B("k", 1, *a_k1)
            emit_B("q", 1, *a_q1)
            emit_V()

        if DEBUG:
            for t in range(2):
                for hh in range(2):
                    d = nc.dram_tensor(f"dbg_kth{t}{hh}", [128, S], BF16,
                                       kind="ExternalOutput")
                    nc.sync.dma_start(out=d.ap(), in_=kth[t][:, hh, :])
                d = nc.dram_tensor(f"dbg_qt{t}", [128, S], BF16,
                                   kind="ExternalOutput")
                nc.sync.dma_start(out=d.ap(), in_=qt[t][:, :])
                d = nc.dram_tensor(f"dbg_rsT{t}", [128, 32], F32,
                                   kind="ExternalOutput")
                nc.sync.dma_start(out=d.ap(), in_=rsT[t][:, :])

        # ---------------- phase 2: attention (+ interleaved out proj) ---
        with tc.tile_pool(name="ps2", bufs=1, space="PSUM") as ps2:
            from collections import deque
            avq = deque()       # pending av / normalize emitters
            po_units = deque()  # pending out-proj emitters

            def make_po(st, on_act=False, tag="nb"):
                def emit():
                    qh = st // 8
                    po = ps2.tile([128, 1024], F32, tag=tag,
                                  name=f"po{st}")
                    for qc in range(2):
                        for t in range(2):
                            nc.tensor.matmul(
                                po[:, qc * 512:(qc + 1) * 512],
                                vmix[t][qh][:, (st % 8) * 128:(st % 8 + 1) * 128],
                                wout[:, t, qc * 512:(qc + 1) * 512],
                                start=(t == 0), stop=(t == 1))
                    o = outp.tile([128, 1024], BF16, tag="o", name=f"o{st}")
                    if on_act:
                        nc.scalar.copy(o[:, :], po[:, :])
                    else:
                        nc.vector.tensor_copy(o[:, :], po[:, :])
                    nc.sync.dma_start(
                        out=out_d.ap()[st * 128:(st + 1) * 128, :],
                        in_=o[:, :])
                return emit

            sections = [(pair, h, qh) for qh in range(2)
                        for pair in range(2) for h in range(2)]
            for si, (pair, h, qh) in enumerate(sections):
                q0 = qh * 1024
                head = 2 * pair + h
                ps_sc = [ps2.tile([128, 1024], F32, tag=f"sc{ab}",
                                  name=f"sc{si}_{ab}") for ab in range(2)]
                pav = ps2.tile([65, 1024], F32, tag="av", name=f"av{si}")
                es = {}
                for kt in range(16):
                    pssc = ps_sc[kt % 2]
                    for qc in range(2):
                        nc.tensor.matmul(
                            pssc[:, qc * 512:(qc + 1) * 512],
                            kth[pair][:, h, kt * 128:(kt + 1) * 128],
                            qt[pair][:, q0 + qc * 512:q0 + (qc + 1) * 512],
                            start=True, stop=True)
                    e = expp.tile([128, 1024], BF16, tag=f"e{kt % 4}",
                                  name=f"e{si}_{kt}")
                    nc.scalar.activation(
                        e[:, :], pssc[:, :], AF.Exp,
                        scale=rsT[pair][:, 2 * kt + h:2 * kt + h + 1])
                    es[kt] = e
                    # drain one pending unit (av lags by ~2 slots)
                    if kt == 6 and po_units:
                        po_units.popleft()()
                    if avq:
                        avq.popleft()()
                        if kt == 3 and avq:   # catch up after normalize slot
                            avq.popleft()()
                    if kt == 11 and po_units:
                        po_units.popleft()()

                    def make_av(kt, pav=pav, es=es, head=head, si=si):
                        def emit():
                            ek = es[kt]
                            for qc in range(2):
                                nc.tensor.matmul(
                                    pav[:, qc * 512:(qc + 1) * 512],
                                    vhat[kt // 4][:, kt % 4, head, :],
                                    ek[:, qc * 512:(qc + 1) * 512],
                                    start=(kt == 0), stop=(kt == 15),
                                    skip_group_check=True)
                        return emit
                    if kt >= 2:
                        make_av(kt - 2)()
                # leftovers: av(14), av(15), then normalize
                avq.append(make_av(14))
                avq.append(make_av(15))

                def make_norm(pair=pair, h=h, qh=qh, pav=pav, si=si):
                    def emit():
                        se = tmp.tile([1, 1024], F32, tag="se",
                                      name=f"se{si}")
                        nc.vector.tensor_copy(se[:, :], pav[64:65, :])
                        rc = tmp.tile([1, 1024], BF16, tag="rc",
                                      name=f"rc{si}")
                        from concourse.dve_ops import (
                            RECIP_APPROX_FAST_CONSTS, RECIPROCAL_APPROX_FAST)
                        _c = RECIP_APPROX_FAST_CONSTS
                        nc.vector._custom_dve(RECIPROCAL_APPROX_FAST,
                                              out=rc[:, :],
                                              in0=se[:, :],
                                              s0=_c["s0"], s1=_c["s1"],
                                              imm2=_c["imm2"])
                        nb = ps2.tile([128, 1024], F32, tag="nb",
                                      name=f"nb{si}")
                        for qc in range(2):
                            nc.tensor.matmul(nb[:, qc * 512:(qc + 1) * 512],
                                             sel2[:, h, :],
                                             rc[:, qc * 512:(qc + 1) * 512],
                                             start=True, stop=True)
                        avs = tmp.tile([64, 1024], F32, tag="avs",
                                       name=f"avs{si}")
                        nc.vector.tensor_copy(avs[:, :], pav[0:64, :])
                        nc.vector.tensor_mul(
                            vmix[pair][qh][h * 64:(h + 1) * 64, :],
                            avs[:, :], nb[h * 64:(h + 1) * 64, :])
                    return emit
                avq.append(make_norm())
                if si == 3:      # vmix[*][qh0] complete after section 3
                    for st in range(8):
                        po_units.append(make_po(st))

            # tail: drain remaining av/normalize, then out proj qh1
            while avq:
                avq.popleft()()
            for st in range(8, 16):
                po_units.append(make_po(st, on_act=(st % 2 == 0),
                                        tag=("sc0", "sc1", "nb")[st % 3]))
            while po_units:
                po_units.popleft()()

            if DEBUG:
                for t in range(2):
                    for qh in range(2):
                        d = nc.dram_tensor(f"dbg_vmix{t}{qh}", [128, 1024],
                                           BF16, kind="ExternalOutput")
                        nc.sync.dma_start(out=d.ap(), in_=vmix[t][qh][:, :])

    nc.compile()
    return nc


def host_prep(x, pos, Wqkv, bqkv, Wout, bout, q_scale, k_scale):
    """Build per-core input maps + shared-table decision."""
    x = np.asarray(x, dtype=np.float32)
    pos = np.asarray(pos, dtype=np.float32).reshape(-1)
    Wqkv = np.asarray(Wqkv, dtype=np.float32)
    bqkv = np.asarray(bqkv, dtype=np.float32)
    Wout = np.asarray(Wout, dtype=np.float32)
    q_scale = np.asarray(q_scale, dtype=np.float32)
    k_scale = np.asarray(k_scale, dtype=np.float32)

    shared = bool(np.array_equal(q_scale, k_scale))
    exp_scale = (1.0 / np.sqrt(DH)) if shared else 1.0

    # rope base tables [128, S]
    i_of_p = (np.arange(128) % 64) // 2            # pair index
    sign = np.where(np.arange(128) % 2 == 0, 1.0, -1.0)
    omega = THETA ** (-np.arange(0, DH, 2, dtype=np.float64) / DH)  # [32]
    ang = pos[None, :].astype(np.float64) * omega[:, None]          # [32, S]
    cosb = np.cos(ang)[i_of_p, :]                  # [128, S]
    sinb = np.sin(ang)[i_of_p, :] * sign[:, None]

    def tables(scale_vec, extra):
        sv = np.tile(scale_vec, 2)                 # [128]
        svx = np.tile(scale_vec[np.arange(64) ^ 1], 2)
        cosT = (cosb * sv[:, None] * extra).astype(np.float32)
        sinT = (sinb * svx[:, None] * extra).astype(np.float32)
        return np.ascontiguousarray(cosT), np.ascontiguousarray(sinT)

    cos_k, sin_k = tables(k_scale, 1.0)
    if not shared:
        cos_q, sin_q = tables(q_scale, 1.0 / np.sqrt(DH))

    bf = ml_dtypes.bfloat16
    Pm = np.zeros((128, 128), dtype=bf)
    Pm[np.arange(128), np.arange(128) ^ 1] = 1.0
    onesblk = np.zeros((128, 2), dtype=bf)
    onesblk[0:64, 0] = 1.0
    onesblk[64:128, 1] = 1.0
    # q-path sumsq gather: [8,512] rows (2*sc, 2*sc+1) = head halves
    onesblk8 = np.zeros((128, 4, 8), dtype=bf)
    ones2blk8 = np.zeros((8, 4, 128), dtype=np.float32)
    for sc in range(4):
        onesblk8[0:64, sc, 2 * sc] = 1.0
        onesblk8[64:128, sc, 2 * sc + 1] = 1.0
        ones2blk8[2 * sc, sc, 0:64] = 1.0
        ones2blk8[2 * sc + 1, sc, 64:128] = 1.0
    sel2 = np.zeros((1, 2, 128), dtype=bf)
    sel2[0, 0, 0:64] = 1.0
    sel2[0, 1, 64:128] = 1.0

    in_maps = []
    for c in range(NC):
        b, g = c // 4, c % 4
        xT = np.ascontiguousarray(
            x[b].T.reshape(NDT, 128, S).transpose(1, 0, 2)).astype(bf)
        wq = Wqkv[:, g * HD:(g + 1) * HD]
        wk = Wqkv[:, DM + g * HD: DM + (g + 1) * HD]
        wv = Wqkv[:, 2 * DM + g * HD: 2 * DM + (g + 1) * HD]
        w_all = np.ascontiguousarray(
            np.concatenate([wq, wk, wv], axis=1)
            .reshape(NDT, 128, 3 * HD).transpose(1, 0, 2)).astype(bf)
        wo = np.ascontiguousarray(
            Wout[g * HD:(g + 1) * HD, :]
            .reshape(2, 128, DM).transpose(1, 0, 2)).astype(bf)
        bqs = np.ascontiguousarray(
            bqkv[g * HD:(g + 1) * HD].reshape(2, 128).T)         # [128, 2]
        bks = np.ascontiguousarray(
            bqkv[DM + g * HD: DM + (g + 1) * HD].reshape(2, 128).T)
        m = {"xT": xT, "w_all": w_all, "wout": wo, "bq": bqs, "bk": bks,
             "cos_k": cos_k, "sin_k": sin_k, "Pswap": Pm, "onesblk": onesblk,
             "onesblk8": onesblk8, "ones2blk8": ones2blk8, "sel2": sel2}
        if not shared:
            m["cos_q"] = cos_q
            m["sin_q"] = sin_q
        in_maps.append(m)

    bias_row = (bqkv[2 * DM:] @ Wout + np.asarray(bout, dtype=np.float32)) \
        .astype(np.float32)                                       # [1024]
    return in_maps, shared, float(exp_scale), bias_row


def _install_ntff_shim():
    """Make trace=True usable: this image lacks antenv.axon_hooks; recreate
    it against the baked libaxon_pjrt.so C ABI (no-op if already present)."""
    try:
        from antenv.axon_hooks import get_axon_ntff_profile_hook  # noqa: F401
        return
    except ImportError:
        pass
    try:
        import types, ctypes, contextlib
        import antenv
        lib = ctypes.CDLL("/opt/axon/libaxon_pjrt.so")
        if not hasattr(lib, "axon_start_nrt_profile"):
            raise OSError("no profile symbols")
        lib.axon_start_nrt_profile.argtypes = [ctypes.POINTER(ctypes.c_int64),
                                               ctypes.c_size_t]
        lib.axon_start_nrt_profile.restype = ctypes.c_int64
        lib.axon_stop_nrt_profile.argtypes = [ctypes.c_char_p]
        lib.axon_stop_nrt_profile.restype = ctypes.c_int64

        @contextlib.contextmanager
        def _hook(output_dir, device_ids):
            import jax
            jax.devices()
            if device_ids:
                ids = (ctypes.c_int64 * len(device_ids))(*device_ids)
                rc = lib.axon_start_nrt_profile(ids, len(device_ids))
            else:
                rc = lib.axon_start_nrt_profile(None, 0)
            if rc != 0:
                raise RuntimeError(f"axon_start_nrt_profile rc={rc}")
            try:
                yield
            finally:
                lib.axon_stop_nrt_profile(str(output_dir).encode())

        mod = types.ModuleType("antenv.axon_hooks")
        mod.get_axon_ntff_profile_hook = lambda: _hook
        mod.set_axon_ntff_profile_hook = lambda h: None
        sys.modules["antenv.axon_hooks"] = mod
        antenv.axon_hooks = mod
    except Exception:
        os.environ["BASS_NEVER_TRACE"] = "1"   # degrade: run untraced


def kernel(x, pos, Wqkv, bqkv, Wout, bout, q_scale, k_scale):
    global LAST_RESULTS
    if os.environ.get("BASS_TRACE"):
        _install_ntff_shim()
    in_maps, shared, exp_scale, bias_row = host_prep(
        x, pos, Wqkv, bqkv, Wout, bout, q_scale, k_scale)

    key = (shared, round(exp_scale, 9))
    if key not in _CACHED:
        _CACHED[key] = build_program(exp_scale, shared)
    nc = _CACHED[key]

    res = bass_utils.run_bass_kernel_spmd(
        nc, in_maps, list(range(NC)),
        trace=bool(os.environ.get("BASS_TRACE")))
    LAST_RESULTS = res

    out = np.empty((B, S, DM), dtype=np.float32)
    for b in range(B):
        acc = bias_row[None, :].astype(np.float32).repeat(S, axis=0)
        for g in range(4):
            acc = acc + res.results[b * 4 + g]["outp"].astype(np.float32)
        out[b] = acc
    return out


# revision 29
# speedup vs baseline: 1.2663x; 1.0002x over previous
"""Trainium2 Bass kernel for nn_Attention_32650341384246.

Full attention layer: qkv proj + per-head RMSNorm(q,k) + RoPE + softmax
attention (non-causal) + out proj.  B=2, S=2048, D=1024, H=16, DH=64.

Sharding: 8 cores; core c handles batch c//4, heads [4*(c%4), 4*(c%4)+4)
(data parallel over batch x tensor parallel over heads).  Each core
computes a partial [S, D] output (its heads @ Wout row-slice); the host
sums the 4 partials per batch and adds the (folded) biases.

Device design (per core), v2 (ACT-exp roofline shape):
  - qkv proj emits qT/kT head-major [128 (2 heads x 64), S] (lhsT = W
    slice, rhs = xT slice) and v s-major [s, 4*64].
  - q-path RMSNorm: sumsq via ones-block matmul into a shared [8,512]
    PSUM tile (one Ln+Exp per section), partition-broadcast via
    ones-matmul, u = tt*pb on DVE.
  - k-path RMSNorm is FOLDED INTO THE SOFTMAX EXP: sumsq is computed
    transposed ([s-part, head] via lhsT=sq-slice matmuls into [128,32]
    PSUM), one Ln+Exp gives rsT = exp_scale*rsqrt(ms+eps), and the
    attention exp uses scale=rsT[:, 2*kt+h] (per-partition AP).  The
    k rope output is left unnormalized (rsqrt commutes past rope).
  - RoPE as rot = cosT*u + sinT'*swap(u); swap = adjacent-partition
    permutation matmul (bf16); tables host-built with scales folded.
  - attention: 8 sections (qh, pair, h); per kt: one [128,1024] score
    matmul (double-buffered PSUM scA/scB), one [128,1024] exp, one
    accumulating [65,1024] AV matmul (row 64 = ones = sumexp).  The
    emission is a flat software-pipelined stream: av lags sc/exp by 2
    slots, section normalize (reciprocal + K=1 broadcast matmul +
    vmix mul) is deferred into the next section's slots, so ACT stays
    saturated across section boundaries.
  - out proj: interleaved into the last attention sections (PSUM tag
    shared with the normalize broadcast), copies on DVE, bf16 out.
ACT runs ONLY Exp/Ln (single table set, no reloads).
"""
import sys, os

sys.path.insert(0, "/opt/trn_rl_repo")

import math
import numpy as np
from contextlib import ExitStack

import ml_dtypes
import concourse.bass as bass
import concourse.mybir as mybir
import concourse.tile as tile
from concourse import bacc
from concourse import bass_utils

F32 = mybir.dt.float32
F32R = mybir.dt.float32r
BF16 = mybir.dt.bfloat16
AF = mybir.ActivationFunctionType

B, S, DM, H, DH = 2, 2048, 1024, 16, 64
NC = 8
HPC = H // 4          # 4 heads per core
HD = HPC * DH         # 256
NDT = DM // 128       # 8 model-dim tiles
THETA, EPS = 10000.0, 1e-6

LAST_RESULTS = None   # BassKernelResults of the most recent device run
_CACHED = {}
DEBUG = False         # add debug DMA taps (qt/kt/rsT) for CoreSim bisection


def build_program(exp_scale: float, shared_tables: bool):
    nc = bacc.Bacc("TRN2", target_bir_lowering=False, debug=False)

    # All ACT funcs we use (Exp/Ln/Identity/Square/Copy) coexist in the
    # 'natural_log_exp_and_others' set, but the table-load placement pass
    # first-matches each func against the set list (Exp->set0, Ln->set5),
    # reloading tables at every rsqrt.  Strip our funcs from the other
    # (cached) set entries so every instruction resolves to the one shared
    # set => a single ACT_TABLE_LOAD for the whole kernel.
    from concourse.hw_specs import get_activation_tables
    _tabs = get_activation_tables(nc.m.arch)
    _ours = {AF.Exp, AF.Ln, AF.Identity, AF.Square, AF.Copy}
    if "natural_log_exp_and_others" in _tabs and             _ours <= _tabs["natural_log_exp_and_others"]:
        for _name, _s in _tabs.items():
            if _name != "natural_log_exp_and_others":
                _s -= _ours

    xT_d = nc.dram_tensor("xT", [128, NDT, S], BF16, kind="ExternalInput")
    w_d = nc.dram_tensor("w_all", [128, NDT, 3 * HD], BF16, kind="ExternalInput")
    wout_d = nc.dram_tensor("wout", [128, 2, DM], BF16, kind="ExternalInput")
    bq_d = nc.dram_tensor("bq", [128, 2], F32, kind="ExternalInput")
    bk_d = nc.dram_tensor("bk", [128, 2], F32, kind="ExternalInput")
    cosk_d = nc.dram_tensor("cos_k", [128, S], F32, kind="ExternalInput")
    sink_d = nc.dram_tensor("sin_k", [128, S], F32, kind="ExternalInput")
    if not shared_tables:
        cosq_d = nc.dram_tensor("cos_q", [128, S], F32, kind="ExternalInput")
        sinq_d = nc.dram_tensor("sin_q", [128, S], F32, kind="ExternalInput")
    P_d = nc.dram_tensor("Pswap", [128, 128], BF16, kind="ExternalInput")
    ob_d = nc.dram_tensor("onesblk", [128, 2], BF16, kind="ExternalInput")
    ob8_d = nc.dram_tensor("onesblk8", [128, 4, 8], BF16, kind="ExternalInput")
    o2b8_d = nc.dram_tensor("ones2blk8", [8, 4, 128], F32R, kind="ExternalInput")
    sel2_d = nc.dram_tensor("sel2", [1, 2, 128], BF16, kind="ExternalInput")
    out_d = nc.dram_tensor("outp", [S, DM], BF16, kind="ExternalOutput")

    ln_exp_bias = float(math.log(exp_scale)) if exp_scale != 1.0 else 0.0

    with tile.TileContext(nc) as tc, ExitStack() as ctx, \
            nc.allow_low_precision(reason="bf16 matmul inputs"):
        singles = ctx.enter_context(tc.tile_pool(name="singles", bufs=1))
        tmp = ctx.enter_context(tc.tile_pool(name="tmp", bufs=2))
        expp = ctx.enter_context(tc.tile_pool(name="expp", bufs=2))
        outp = ctx.enter_context(tc.tile_pool(name="outp", bufs=4))

        # --- first-needed loads up front; per-dt tiles so Tile's
        # per-tile RAW tracking doesn't serialize readers behind all DMAs ---
        w_dt = [singles.tile([128, 3 * HD], BF16, name=f"w{dt}") for dt in range(NDT)]
        x_dt = [singles.tile([128, S], BF16, name=f"x{dt}") for dt in range(NDT)]
        for dt in range(NDT):
            nc.sync.dma_start(out=w_dt[dt], in_=w_d.ap()[:, dt, :])
            nc.sync.dma_start(out=x_dt[dt], in_=xT_d.ap()[:, dt, :])

        bq = singles.tile([128, 2], F32)
        nc.sync.dma_start(out=bq, in_=bq_d.ap())
        bk = singles.tile([128, 2], F32)
        nc.sync.dma_start(out=bk, in_=bk_d.ap())
        Pm = singles.tile([128, 128], BF16)
        nc.sync.dma_start(out=Pm, in_=P_d.ap())
        onesblk = singles.tile([128, 2], BF16)
        nc.sync.dma_start(out=onesblk, in_=ob_d.ap())
        onesblk8 = singles.tile([128, 4, 8], BF16)
        nc.sync.dma_start(out=onesblk8, in_=ob8_d.ap())
        ones2blk8 = singles.tile([8, 4, 128], F32R)
        nc.sync.dma_start(out=ones2blk8, in_=o2b8_d.ap())
        sel2 = singles.tile([1, 2, 128], BF16)
        nc.sync.dma_start(out=sel2, in_=sel2_d.ap())
        cos_k = singles.tile([128, S], F32)
        nc.sync.dma_start(out=cos_k, in_=cosk_d.ap())
        sin_k = singles.tile([128, S], F32)
        nc.sync.dma_start(out=sin_k, in_=sink_d.ap())
        if shared_tables:
            cos_q, sin_q = cos_k, sin_k
        else:
            cos_q = singles.tile([128, S], F32)
            nc.sync.dma_start(out=cos_q, in_=cosq_d.ap())
            sin_q = singles.tile([128, S], F32)
            nc.sync.dma_start(out=sin_q, in_=sinq_d.ap())
        wout = singles.tile([128, 2, DM], BF16)
        nc.sync.dma_start(out=wout, in_=wout_d.ap())
        eps8 = singles.tile([8, 1], F32)
        nc.vector.memset(eps8, EPS)
        eps128 = singles.tile([128, 1], F32)
        nc.vector.memset(eps128, EPS)
        lnb128 = singles.tile([128, 1], F32)
        nc.vector.memset(lnb128, ln_exp_bias)

        # head-major roped q: [128 (2 heads x 64 dims), 2048 s] per pair
        qt = [singles.tile([128, S], BF16, name=f"qt{t}") for t in range(2)]
        # k zero-padded per head: kth[t][:, h, :] has head-h dims in rows
        # 64h..64h+63, zeros elsewhere -> K=128 score matmuls, no PE
        # row-tiling mode switches against the K=128 AV matmuls.
        kth = [singles.tile([128, 2, S], BF16, name=f"kth{t}") for t in range(2)]
        for t in range(2):
            nc.gpsimd.memset(kth[t][:, :, :], 0.0)
        # rsT[t][:, 2*kt+h] = exp_scale * rsqrt(mean k^2 + eps) per k-pos
        rsT = [singles.tile([128, 32], F32, name=f"rsT{t}") for t in range(2)]
        vhat = [singles.tile([128, 4, HPC, 65], BF16, name=f"vhat{sc}")
                for sc in range(4)]
        for sc in range(4):
            nc.vector.memset(vhat[sc][:, :, :, 64:65], 1.0)
        vmix = [[singles.tile([128, 1024], BF16, name=f"vmix{t}_{qh}")
                 for qh in range(2)] for t in range(2)]

        # ---------------- phase 1: qkv + rmsnorm + rope ----------------
        with tc.tile_pool(name="ps1", bufs=1, space="PSUM") as ps1:
            def emit_A(which, t):
                """proj + bias + square + sumsq (+ k: transposed rsqrt)."""
                off = 0 if which == "q" else HD
                bias = bq if which == "q" else bk
                tts = []
                if which == "q":
                    pss = ps1.tile([8, 512], F32, tag="pss",
                                   name=f"pss{which}{t}")
                else:
                    rstp = ps1.tile([128, 32], F32, tag="rstp",
                                    name=f"rstp{t}")
                for sc in range(4):
                    s0 = sc * 512
                    pq = ps1.tile([128, 512], F32, tag="pq", bufs=2,
                                  name=f"pq{which}{t}_{sc}")
                    for dt in range(NDT):
                        nc.tensor.matmul(
                            pq[:, :],
                            w_dt[dt][:, off + t * 128: off + (t + 1) * 128],
                            x_dt[dt][:, s0:s0 + 512],
                            start=(dt == 0), stop=(dt == NDT - 1))
                    tt = tmp.tile([128, 512], BF16, tag="tt", bufs=10,
                                  name=f"tt{which}{t}_{sc}")
                    nc.scalar.activation(tt[:, :], pq[:, :], AF.Identity,
                                         bias=bias[:, t:t + 1], scale=1.0)
                    tts.append(tt)
                    sq = tmp.tile([128, 512], BF16, tag="sq", bufs=3,
                                  name=f"sq{which}{t}_{sc}")
                    nc.scalar.activation(sq[:, :], pq[:, :], AF.Square,
                                         bias=bias[:, t:t + 1], scale=1.0)
                    if which == "q":
                        nc.tensor.matmul(pss[:, :], onesblk8[:, sc, :],
                                         sq[:, :],
                                         start=(sc == 0), stop=(sc == 3))
                    else:
                        for qtr in range(4):
                            kt = 4 * sc + qtr
                            nc.tensor.matmul(
                                rstp[:, 2 * kt:2 * kt + 2],
                                sq[:, qtr * 128:(qtr + 1) * 128],
                                onesblk[:, :],
                                start=True, stop=True)
                if which == "q":
                    lns = tmp.tile([8, 512], F32, tag="lns",
                                   name=f"lns{which}{t}")
                    nc.scalar.activation(lns[:, :], pss[:, :], AF.Ln,
                                         bias=eps8[:, :], scale=1.0 / DH)
                    rs = tmp.tile([8, 512], F32R, tag="rs",
                                  name=f"rs{which}{t}")
                    nc.scalar.activation(rs[:, :], lns[:, :], AF.Exp,
                                         scale=-0.5)
                    return tts, rs
                else:
                    lnt = tmp.tile([128, 32], F32, tag="lnt",
                                   name=f"lnt{t}")
                    nc.scalar.activation(lnt[:, :], rstp[:, :], AF.Ln,
                                         bias=eps128[:, :], scale=1.0 / DH)
                    nc.scalar.activation(rsT[t][:, :], lnt[:, :], AF.Exp,
                                         scale=-0.5, bias=lnb128[:, :])
                    return tts, None

            def emit_B(which, t, tts, rs):
                """(q: normalize) + rope -> qt/kth tiles."""
                cosT = cos_q if which == "q" else cos_k
                sinT = sin_q if which == "q" else sin_k
                for sc in range(4):
                    s0 = sc * 512
                    if which == "q":
                        pb = ps1.tile([128, 512], F32, tag="pb",
                                      name=f"pb{which}{t}_{sc}")
                        nc.tensor.matmul(pb[:, :], ones2blk8[:, sc, :],
                                         rs[:, :], start=True, stop=True)
                        u = tmp.tile([128, 512], BF16, tag="u", bufs=3,
                                     name=f"u{which}{t}_{sc}")
                        nc.vector.tensor_mul(u[:, :], tts[sc][:, :], pb[:, :])
                    else:
                        u = tts[sc]
                    psw = ps1.tile([128, 512], F32, tag="psw",
                                   name=f"psw{which}{t}_{sc}")
                    nc.tensor.matmul(psw[:, :], Pm[:, :], u[:, :],
                                     start=True, stop=True)
                    t1 = tmp.tile([128, 512], F32, tag="t1", bufs=3,
                                  name=f"t1{which}{t}_{sc}")
                    nc.gpsimd.tensor_mul(t1[:, :], u[:, :],
                                         cosT[:, s0:s0 + 512])
                    t2 = tmp.tile([128, 512], F32, tag="t2", bufs=3,
                                  name=f"t2{which}{t}_{sc}")
                    nc.vector.tensor_mul(t2[:, :], psw[:, :],
                                         sinT[:, s0:s0 + 512])
                    if which == "q":
                        nc.gpsimd.tensor_add(qt[t][:, s0:s0 + 512],
                                             t1[:, :], t2[:, :])
                    else:
                        for hh in range(2):
                            nc.gpsimd.tensor_add(
                                kth[t][hh * 64:(hh + 1) * 64, hh,
                                       s0:s0 + 512],
                                t1[hh * 64:(hh + 1) * 64, :],
                                t2[hh * 64:(hh + 1) * 64, :])

            def emit_V():
                for sc in range(4):
                    for st in range(4):
                        pv = ps1.tile([128, HD], F32, tag="pv", bufs=2,
                                      name=f"pv{sc}_{st}")
                        for dt in range(NDT):
                            nc.tensor.matmul(
                                pv[:, :],
                                x_dt[dt][:, sc * 512 + st * 128:
                                         sc * 512 + (st + 1) * 128],
                                w_dt[dt][:, 2 * HD:3 * HD],
                                start=(dt == 0), stop=(dt == NDT - 1))
                        nc.scalar.copy(
                            vhat[sc][:, st, :, 0:64],
                            pv[:, :].rearrange("p (h d) -> p h d", h=HPC))

            # A/B software pipeline: B(x) consumes rsqrt computed during
            # the next A's matmuls, so PE never stalls on ACT.
            a_k0 = emit_A("k", 0)
            a_q0 = emit_A("q", 0)
            emit_B("k", 0, *a_k0)
            a_k1 = emit_A("k", 1)
            emit_B("q", 0, *a_q0)
            a_q1 = emit_A("q", 1)
            emit_B("k", 1, *a_k1)
            emit_B("q", 1, *a_q1)
            emit_V()

        if DEBUG:
            for t in range(2):
                for hh in range(2):
                    d = nc.dram_tensor(f"dbg_kth{t}{hh}", [128, S], BF16,
                                       kind="ExternalOutput")
                    nc.sync.dma_start(out=d.ap(), in_=kth[t][:, hh, :])
                d = nc.dram_tensor(f"dbg_qt{t}", [128, S], BF16,
                                   kind="ExternalOutput")
                nc.sync.dma_start(out=d.ap(), in_=qt[t][:, :])
                d = nc.dram_tensor(f"dbg_rsT{t}", [128, 32], F32,
                                   kind="ExternalOutput")
                nc.sync.dma_start(out=d.ap(), in_=rsT[t][:, :])

        # ---------------- phase 2: attention (+ interleaved out proj) ---
        with tc.tile_pool(name="ps2", bufs=1, space="PSUM") as ps2:
            from collections import deque
            avq = deque()       # pending av / normalize emitters
            po_units = deque()  # pending out-proj emitters

            def make_po(st, on_act=False, tag="nb"):
                def emit():
                    qh = st // 8
                    po = ps2.tile([128, 1024], F32, tag=tag,
                                  name=f"po{st}")
                    for qc in range(2):
                        for t in range(2):
                            nc.tensor.matmul(
                                po[:, qc * 512:(qc + 1) * 512],
                                vmix[t][qh][:, (st % 8) * 128:(st % 8 + 1) * 128],
                                wout[:, t, qc * 512:(qc + 1) * 512],
                                start=(t == 0), stop=(t == 1))
                    o = outp.tile([128, 1024], BF16, tag="o", name=f"o{st}")
                    if on_act:
                        nc.scalar.copy(o[:, :], po[:, :])
                    else:
                        nc.vector.tensor_copy(o[:, :], po[:, :])
                    nc.sync.dma_start(
                        out=out_d.ap()[st * 128:(st + 1) * 128, :],
                        in_=o[:, :])
                return emit

            sections = [(pair, h, qh) for qh in range(2)
                        for pair in range(2) for h in range(2)]
            for si, (pair, h, qh) in enumerate(sections):
                q0 = qh * 1024
                head = 2 * pair + h
                ps_sc = [ps2.tile([128, 1024], F32, tag=f"sc{ab}",
                                  name=f"sc{si}_{ab}") for ab in range(2)]
                pav = ps2.tile([65, 1024], F32, tag="av", name=f"av{si}")
                es = {}
                for kt in range(16):
                    pssc = ps_sc[kt % 2]
                    for qc in range(2):
                        nc.tensor.matmul(
                            pssc[:, qc * 512:(qc + 1) * 512],
                            kth[pair][:, h, kt * 128:(kt + 1) * 128],
                            qt[pair][:, q0 + qc * 512:q0 + (qc + 1) * 512],
                            start=True, stop=True)
                    e = expp.tile([128, 1024], BF16, tag=f"e{kt % 4}",
                                  name=f"e{si}_{kt}")
                    nc.scalar.activation(
                        e[:, :], pssc[:, :], AF.Exp,
                        scale=rsT[pair][:, 2 * kt + h:2 * kt + h + 1])
                    es[kt] = e
                    # drain one pending unit (av lags by ~2 slots)
                    if kt == 6 and po_units:
                        po_units.popleft()()
                    if avq:
                        avq.popleft()()
                        if kt == 3 and avq:   # catch up after normalize slot
                            avq.popleft()()
                    if kt == 11 and po_units:
                        po_units.popleft()()

                    def make_av(kt, pav=pav, es=es, head=head, si=si):
                        def emit():
                            ek = es[kt]
                            for qc in range(2):
                                nc.tensor.matmul(
                                    pav[:, qc * 512:(qc + 1) * 512],
                                    vhat[kt // 4][:, kt % 4, head, :],
                                    ek[:, qc * 512:(qc + 1) * 512],
                                    start=(kt == 0), stop=(kt == 15),
                                    skip_group_check=True)
                        return emit
                    if kt >= 2:
                        make_av(kt - 2)()
                # leftovers: av(14), av(15), then normalize
                avq.append(make_av(14))
                avq.append(make_av(15))

                def make_norm(pair=pair, h=h, qh=qh, pav=pav, si=si):
                    def emit():
                        se = tmp.tile([1, 1024], F32, tag="se",
                                      name=f"se{si}")
                        nc.vector.tensor_copy(se[:, :], pav[64:65, :])
                        rc = tmp.tile([1, 1024], BF16, tag="rc",
                                      name=f"rc{si}")
                        from concourse.dve_ops import (
                            RECIP_APPROX_FAST_CONSTS, RECIPROCAL_APPROX_FAST)
                        _c = RECIP_APPROX_FAST_CONSTS
                        nc.vector._custom_dve(RECIPROCAL_APPROX_FAST,
                                              out=rc[:, :],
                                              in0=se[:, :],
                                              s0=_c["s0"], s1=_c["s1"],
                                              imm2=_c["imm2"])
                        nb = ps2.tile([128, 1024], F32, tag="nb",
                                      name=f"nb{si}")
                        for qc in range(2):
                            nc.tensor.matmul(nb[:, qc * 512:(qc + 1) * 512],
                                             sel2[:, h, :],
                                             rc[:, qc * 512:(qc + 1) * 512],
                                             start=True, stop=True)
                        avs = tmp.tile([64, 1024], F32, tag="avs",
                                       name=f"avs{si}")
                        nc.vector.tensor_copy(avs[:, :], pav[0:64, :])
                        nc.vector.tensor_mul(
                            vmix[pair][qh][h * 64:(h + 1) * 64, :],
                            avs[:, :], nb[h * 64:(h + 1) * 64, :])
                    return emit
                avq.append(make_norm())
                if si == 3:      # vmix[*][qh0] complete after section 3
                    for st in range(8):
                        po_units.append(make_po(st))

            # tail: drain remaining av/normalize, then out proj qh1
            while avq:
                avq.popleft()()
            for st in range(8, 16):
                po_units.append(make_po(st, on_act=(st % 2 == 0),
                                        tag=("sc0", "sc1", "nb")[st % 3]))
            while po_units:
                po_units.popleft()()

            if DEBUG:
                for t in range(2):
                    for qh in range(2):
                        d = nc.dram_tensor(f"dbg_vmix{t}{qh}", [128, 1024],
                                           BF16, kind="ExternalOutput")
                        nc.sync.dma_start(out=d.ap(), in_=vmix[t][qh][:, :])

    nc.compile()
    return nc


def host_prep(x, pos, Wqkv, bqkv, Wout, bout, q_scale, k_scale):
    """Build per-core input maps + shared-table decision."""
    x = np.asarray(x, dtype=np.float32)
    pos = np.asarray(pos, dtype=np.float32).reshape(-1)
    Wqkv = np.asarray(Wqkv, dtype=np.float32)
    bqkv = np.asarray(bqkv, dtype=np.float32)
    Wout = np.asarray(Wout, dtype=np.float32)
    q_scale = np.asarray(q_scale, dtype=np.float32)
    k_scale = np.asarray(k_scale, dtype=np.float32)

    shared = bool(np.array_equal(q_scale, k_scale))
    exp_scale = (1.0 / np.sqrt(DH)) if shared else 1.0

    # rope base tables [128, S]
    i_of_p = (np.arange(128) % 64) // 2            # pair index
    sign = np.where(np.arange(128) % 2 == 0, 1.0, -1.0)
    omega = THETA ** (-np.arange(0, DH, 2, dtype=np.float64) / DH)  # [32]
    ang = pos[None, :].astype(np.float64) * omega[:, None]          # [32, S]
    cosb = np.cos(ang)[i_of_p, :]                  # [128, S]
    sinb = np.sin(ang)[i_of_p, :] * sign[:, None]

    def tables(scale_vec, extra):
        sv = np.tile(scale_vec, 2)                 # [128]
        svx = np.tile(scale_vec[np.arange(64) ^ 1], 2)
        cosT = (cosb * sv[:, None] * extra).astype(np.float32)
        sinT = (sinb * svx[:, None] * extra).astype(np.float32)
        return np.ascontiguousarray(cosT), np.ascontiguousarray(sinT)

    cos_k, sin_k = tables(k_scale, 1.0)
    if not shared:
        cos_q, sin_q = tables(q_scale, 1.0 / np.sqrt(DH))

    bf = ml_dtypes.bfloat16
    Pm = np.zeros((128, 128), dtype=bf)
    Pm[np.arange(128), np.arange(128) ^ 1] = 1.0
    onesblk = np.zeros((128, 2), dtype=bf)
    onesblk[0:64, 0] = 1.0
    onesblk[64:128, 1] = 1.0
    # q-path sumsq gather: [8,512] rows (2*sc, 2*sc+1) = head halves
    onesblk8 = np.zeros((128, 4, 8), dtype=bf)
    ones2blk8 = np.zeros((8, 4, 128), dtype=np.float32)
    for sc in range(4):
        onesblk8[0:64, sc, 2 * sc] = 1.0
        onesblk8[64:128, sc, 2 * sc + 1] = 1.0
        ones2blk8[2 * sc, sc, 0:64] = 1.0
        ones2blk8[2 * sc + 1, sc, 64:128] = 1.0
    sel2 = np.zeros((1, 2, 128), dtype=bf)
    sel2[0, 0, 0:64] = 1.0
    sel2[0, 1, 64:128] = 1.0

    in_maps = []
    for c in range(NC):
        b, g = c // 4, c % 4
        xT = np.ascontiguousarray(
            x[b].T.reshape(NDT, 128, S).transpose(1, 0, 2)).astype(bf)
        wq = Wqkv[:, g * HD:(g + 1) * HD]
        wk = Wqkv[:, DM + g * HD: DM + (g + 1) * HD]
        wv = Wqkv[:, 2 * DM + g * HD: 2 * DM + (g + 1) * HD]
        w_all = np.ascontiguousarray(
            np.concatenate([wq, wk, wv], axis=1)
            .reshape(NDT, 128, 3 * HD).transpose(1, 0, 2)).astype(bf)
        wo = np.ascontiguousarray(
            Wout[g * HD:(g + 1) * HD, :]
            .reshape(2, 128, DM).transpose(1, 0, 2)).astype(bf)
        bqs = np.ascontiguousarray(
            bqkv[g * HD:(g + 1) * HD].reshape(2, 128).T)         # [128, 2]
        bks = np.ascontiguousarray(
            bqkv[DM + g * HD: DM + (g + 1) * HD].reshape(2, 128).T)
        m = {"xT": xT, "w_all": w_all, "wout": wo, "bq": bqs, "bk": bks,
             "cos_k": cos_k, "sin_k": sin_k, "Pswap": Pm, "onesblk": onesblk,
             "onesblk8": onesblk8, "ones2blk8": ones2blk8, "sel2": sel2}
        if not shared:
            m["cos_q"] = cos_q
            m["sin_q"] = sin_q
        in_maps.append(m)

    bias_row = (bqkv[2 * DM:] @ Wout + np.asarray(bout, dtype=np.float32)) \
        .astype(np.float32)                                       # [1024]
    return in_maps, shared, float(exp_scale), bias_row


def _install_ntff_shim():
    """Make trace=True usable: this image lacks antenv.axon_hooks; recreate
    it against the baked libaxon_pjrt.so C ABI (no-op if already present)."""
    try:
        from antenv.axon_hooks import get_axon_ntff_profile_hook  # noqa: F401
        return
    except ImportError:
        pass
    try:
        import types, ctypes, contextlib
        import antenv
        lib = ctypes.CDLL("/opt/axon/libaxon_pjrt.so")
        if not hasattr(lib, "axon_start_nrt_profile"):
            raise OSError("no profile symbols")
        lib.axon_start_nrt_profile.argtypes = [ctypes.POINTER(ctypes.c_int64),
                                               ctypes.c_size_t]
        lib.axon_start_nrt_profile.restype = ctypes.c_int64
        lib.axon_stop_nrt_profile.argtypes = [ctypes.c_char_p]
        lib.axon_stop_nrt_profile.restype = ctypes.c_int64

        @contextlib.contextmanager
        def _hook(output_dir, device_ids):
            import jax
            jax.devices()
            if device_ids:
                ids = (ctypes.c_int64 * len(device_ids))(*device_ids)
                rc = lib.axon_start_nrt_profile(ids, len(device_ids))
            else:
                rc = lib.axon_start_nrt_profile(None, 0)
            if rc != 0:
                raise RuntimeError(f"axon_start_nrt_profile rc={rc}")
            try:
                yield
            finally:
                lib.axon_stop_nrt_profile(str(output_dir).encode())

        mod = types.ModuleType("antenv.axon_hooks")
        mod.get_axon_ntff_profile_hook = lambda: _hook
        mod.set_axon_ntff_profile_hook = lambda h: None
        sys.modules["antenv.axon_hooks"] = mod
        antenv.axon_hooks = mod
    except Exception:
        os.environ["BASS_NEVER_TRACE"] = "1"   # degrade: run untraced


def kernel(x, pos, Wqkv, bqkv, Wout, bout, q_scale, k_scale):
    global LAST_RESULTS
    if os.environ.get("BASS_TRACE"):
        _install_ntff_shim()
    in_maps, shared, exp_scale, bias_row = host_prep(
        x, pos, Wqkv, bqkv, Wout, bout, q_scale, k_scale)

    key = (shared, round(exp_scale, 9))
    if key not in _CACHED:
        _CACHED[key] = build_program(exp_scale, shared)
    nc = _CACHED[key]

    res = bass_utils.run_bass_kernel_spmd(
        nc, in_maps, list(range(NC)),
        trace=bool(os.environ.get("BASS_TRACE")))
    LAST_RESULTS = res

    out = np.empty((B, S, DM), dtype=np.float32)
    for b in range(B):
        acc = bias_row[None, :].astype(np.float32).repeat(S, axis=0)
        for g in range(4):
            acc = acc + res.results[b * 4 + g]["outp"].astype(np.float32)
        out[b] = acc
    return out
